# revision 21
# baseline (speedup 1.0000x reference)
"""GVSL loss (NCC + MSE + smoothness) as a distributed Bass kernel on 8 TRN2 cores.

Sharding: batch(2) x depth-quarters(4) = 8 shards; each core owns a 32-deep
output slab (+4-voxel halo for the 9^3 box filter).

NCC box filter strategy (per var in {I, J, I^2, J^2, IJ}):
  pass1 (PE):  per d-row matmul(lhsT=V_d[h,w], rhs=BandH[h,h']) -> PSUM [w, h]
               = H-box + transpose in one shot (fp16, FD=128)
  evac1:       PSUM -> SBUF fp16 YT [w, (d, h)]   (rotating DVE/ACT)
  pass2 (PE):  stationary BandW (scaled); 3 d-shifted FD=512 matmuls accumulate
               -> t3[r] = Z[r]+Z[r+1]+Z[r+2] (W-box + D-triple), PSUM
  evac2:       PSUM -> SBUF fp16 T3
  D-final:     S = t3[d] + t3[d+3] + t3[d+6]  (GPSIMD for var J, DVE otherwise)

cc math: cc = (crossS * rsqrt(IvarS*JvarS + eps))^2 -- Rsqrt+Square+Copy all
live in the reciprocal_sqrt_and_small ACT table set: one table load total.

Smoothness loss entirely on the PE via Gram matrices: for flow tiles
X_r = [h, w] per (channel, depth-row),
  sum dz^2 (w-diff)  = <Mq, G>        with G  = sum X_r^T X_r
  sum dy^2 (d-diff)  = <2I, G> - <I, A> - <I, B> - 2<I, C>,
                       C = sum X_r^T X_{r+1}, A = sum_c G_0, B = sum_c G_32
  sum dx^2 (h-diff)  = <Mq, G'>       with G' = sum X'_r^T X'_r (transposed)
All Gram matmuls accumulate into two PSUM banks; masked reduction is a single
fused DVE tensor_tensor_reduce per bank. Mq = quadratic diff mask
(diag [1,2,...,2,1], off-diag -1).

MSE: GPSIMD sub + fused scalar_tensor_tensor square-accumulate.
"""

import os
import sys

for _p in ("/opt/trn_rl_repo",):
    if _p not in sys.path:
        sys.path.insert(0, _p)

import numpy as np

import concourse.bass as bass
import concourse.tile as tile
from concourse import bacc, mybir
from concourse.bass_utils import run_bass_kernel_spmd

F32 = mybir.dt.float32
F16 = mybir.dt.float16
AF = mybir.ActivationFunctionType
ALU = mybir.AluOpType

HP = 128
W = 128
D_FULL = 128
DQ = 32
D_IN = DQ + 8     # 40 slab rows incl halo
YT_R = 40
T3_R = 40
FLOW_D = DQ + 1   # 33 rows for untransposed flow (d-pairs)
FLOWT_D = DQ      # 32 rows for transposed flow

N_IN = D_IN * W           # 5120
N_YT = YT_R * HP          # 5120
N_T3 = T3_R * HP          # 5120
N_BOX = DQ * HP           # 4096
N_RECON = DQ * W          # 4096
N_FLOW_C = FLOW_D * W     # 4224
N_FLOWT_C = FLOWT_D * HP  # 4096
N_MASK = 512 + 128        # 640

S16 = float(np.float16(1.0 / 27.0))          # quadratic-var scale
TLIN = float(np.float16(np.sqrt(S16 / 729.0)))  # linear-var scale
EPS_S = 1e-5 * S16 * S16

NSL = 2                   # cc slices
NS = N_BOX // NSL         # 1024 els per slice

COL_CC = 0     # +NSL
COL_MSE = COL_CC + NSL   # +4
COL_SMG = COL_MSE + 4
COL_SMGP = COL_SMG + 1
ACC_W = 12

VARS = ("J", "I", "II", "JJ", "IJ")

# evacuation engine rotation (PSUM -> SBUF copies): ACT-heavy
EVAC_PAT = ("scalar", "scalar", "vector")

_CACHE = {}

# bisect/debug knobs
NO_GRAM = os.environ.get("GVSL_NO_GRAM", "0") == "1"   # skip smooth Gram MMs + TTRs
NO_TTR = os.environ.get("GVSL_NO_TTR", "1") == "1"     # tensor_tensor_reduce hangs TRN2 HW; keep fallback
NO_GPS = os.environ.get("GVSL_NO_GPS", "0") == "1"     # no gpsimd work (all on DVE)


def _patch_act_tables():
    """Reorder activation-table sets so natural_log_exp_and_others (which
    contains ln + exp + square + copy) is preferred: one ACT table load."""
    from concourse import hw_specs

    if getattr(hw_specs, "_gvsl_patched", False):
        return
    orig = hw_specs.get_activation_tables

    def patched(arch):
        t = dict(orig(arch))
        key = "natural_log_exp_and_others"
        if key in t:
            t = {key: t[key], **{k: v for k, v in t.items() if k != key}}
        return t

    hw_specs.get_activation_tables = patched
    bacc.get_activation_tables = patched
    hw_specs._gvsl_patched = True


def _build_program():
    if os.environ.get("GVSL_PATCH_TABLES", "0") == "1":
        _patch_act_tables()
    nc = bacc.Bacc("TRN2", target_bir_lowering=False, debug=False, num_devices=8)

    d_inI = nc.dram_tensor("inI", [HP, N_IN], F16, kind="ExternalInput").ap()
    d_inJ = nc.dram_tensor("inJ", [HP, N_IN], F16, kind="ExternalInput").ap()
    d_recon = nc.dram_tensor("recon", [HP, N_RECON], F16, kind="ExternalInput").ap()
    d_flow = nc.dram_tensor("flow", [HP, 3 * N_FLOW_C], F16, kind="ExternalInput").ap()
    d_flowT = nc.dram_tensor(
        "flowT", [HP, 3 * N_FLOWT_C], F16, kind="ExternalInput"
    ).ap()
    d_bandh = nc.dram_tensor("bandh", [HP, HP], F16, kind="ExternalInput").ap()
    d_bandq = nc.dram_tensor("bandq", [HP, HP], F16, kind="ExternalInput").ap()
    d_bandl = nc.dram_tensor("bandl", [HP, HP], F16, kind="ExternalInput").ap()
    d_masks = nc.dram_tensor("masks", [HP, N_MASK], F16, kind="ExternalInput").ap()
    d_out = nc.dram_tensor("out", [HP, ACC_W], F32, kind="ExternalOutput").ap()

    from contextlib import ExitStack

    with tile.TileContext(nc) as tc, ExitStack() as es:
        pp = es.enter_context(tc.tile_pool(name="persist", bufs=1))
        prp = es.enter_context(tc.tile_pool(name="prodp", bufs=1))
        ytp = es.enter_context(tc.tile_pool(name="ytp", bufs=2))
        t3p = es.enter_context(tc.tile_pool(name="t3p", bufs=2))
        bxp = es.enter_context(tc.tile_pool(name="boxp", bufs=1))
        scp = es.enter_context(tc.tile_pool(name="scrp", bufs=1))
        flp = es.enter_context(tc.tile_pool(name="flowp", bufs=3))
        ps1 = es.enter_context(tc.tile_pool(name="psum1", bufs=3, space="PSUM"))
        psg = es.enter_context(tc.tile_pool(name="psumG", bufs=1, space="PSUM"))

        acc = pp.tile([HP, ACC_W], F32, tag="acc", name="acc")[:]
        nc.gpsimd.memset(acc, 0.0)
        eps_ap = pp.tile([HP, 1], F32, tag="epsc", name="epsc")[:]
        nc.gpsimd.memset(eps_ap, EPS_S)

        bandh = pp.tile([HP, HP], F16, tag="bandh", name="bandh")[:]
        bandq = pp.tile([HP, HP], F16, tag="bandq", name="bandq")[:]
        bandl = pp.tile([HP, HP], F16, tag="bandl", name="bandl")[:]
        masks = pp.tile([HP, N_MASK], F16, tag="masks", name="masks")[:]
        inI = pp.tile([HP, N_IN], F16, tag="inI", name="inI")[:]
        inJ = pp.tile([HP, N_IN], F16, tag="inJ", name="inJ")[:]
        recon = pp.tile([HP, N_RECON], F16, tag="recon", name="recon")[:]
        gs1 = pp.tile([HP, 512], F32, tag="gs1", name="gs1")[:]
        gs2 = pp.tile([HP, HP], F32, tag="gs2", name="gs2")[:]

        # PSUM Gram accumulators: [G | C | A | B] and G'
        psG = psg.tile([HP, 512], F32, tag="G", name="psG")[:]
        psGp_full = psg.tile([HP, 512], F32, tag="Gp", name="psGp")[:]
        psGp = psGp_full[:, 0:HP]

        NQ = N_IN // 4
        nc.sync.dma_start(out=bandh, in_=d_bandh)
        for q in range(4):
            nc.sync.dma_start(
                out=inJ[:, NQ * q : NQ * (q + 1)],
                in_=d_inJ[:, NQ * q : NQ * (q + 1)],
            )
        for q in range(4):
            nc.sync.dma_start(
                out=inI[:, NQ * q : NQ * (q + 1)],
                in_=d_inI[:, NQ * q : NQ * (q + 1)],
            )
        nc.sync.dma_start(out=bandl, in_=d_bandl)
        nc.sync.dma_start(out=bandq, in_=d_bandq)
        nc.sync.dma_start(out=masks, in_=d_masks)

        # flow channels stream in per-channel ring buffers
        fl_ap = {}
        flt_ap = {}
        for c in range(3):
            t = flp.tile([HP, N_FLOW_C], F16, tag="flc", name=f"flc{c}")[:]
            nc.sync.dma_start(
                out=t, in_=d_flow[:, c * N_FLOW_C : (c + 1) * N_FLOW_C]
            )
            fl_ap[c] = t.rearrange("p (d w) -> p d w", w=W)
        for c in range(3):
            t = flp.tile([HP, N_FLOWT_C], F16, tag="flt", name=f"flt{c}")[:]
            nc.sync.dma_start(
                out=t, in_=d_flowT[:, c * N_FLOWT_C : (c + 1) * N_FLOWT_C]
            )
            flt_ap[c] = t.rearrange("p (d h) -> p d h", h=HP)
        nc.sync.dma_start(out=recon, in_=d_recon)

        inI_r = inI.rearrange("p (d w) -> p d w", w=W)
        inJ_r = inJ.rearrange("p (d w) -> p d w", w=W)

        evac_ct = [0]

        def evac(dst, src):
            eng = EVAC_PAT[evac_ct[0] % len(EVAC_PAT)]
            if eng == "vector":
                nc.vector.tensor_copy(dst, src)
            else:
                nc.scalar.copy(dst, src)
            evac_ct[0] += 1

        # ---- products: JJ full on GPSIMD; IJ, II chunked on DVE ----
        prod_tiles = {}
        srcs = {"J": inJ_r, "I": inI_r}
        for v in ("JJ", "IJ", "II"):
            prod_tiles[v] = prp.tile(
                [HP, N_IN], F16, tag=f"prod{v}", name=f"prod{v}"
            )[:]
            srcs[v] = prod_tiles[v].rearrange("p (d w) -> p d w", w=W)
        if NO_GPS:
            nc.vector.tensor_mul(prod_tiles["JJ"], inJ, inJ)
        else:
            nc.gpsimd.tensor_mul(prod_tiles["JJ"], inJ, inJ)

        def product_chunk(v, c, n=2):
            lo = (N_IN // n) * c
            hi = (N_IN // n) * (c + 1)
            a = inJ if v == "JJ" else inI
            b = inJ if v in ("JJ", "IJ") else inI
            nc.vector.tensor_mul(prod_tiles[v][:, lo:hi], a[:, lo:hi], b[:, lo:hi])

        # ---- pass1 / pass2 ----
        def pass1_chunks(v, src_r, yt_r):
            def mk(g0):
                def emit():
                    pst = ps1.tile([HP, 1024], F32, tag="ps1", name="ps1")[:]
                    for q in range(8):
                        nc.tensor.matmul(
                            pst[:, 128 * q : 128 * (q + 1)],
                            src_r[:, g0 + q, :],
                            bandh,
                            start=True,
                            stop=True,
                        )
                    dst = yt_r[:, g0 : g0 + 8, :].rearrange("p d h -> p (d h)")
                    evac(dst, pst)
                return emit
            return [mk(g0) for g0 in range(0, D_IN, 8)]

        def pass2_chunks(v, yt_r, t3_r):
            bw = bandl if v in ("I", "J") else bandq
            def mk(k0):
                def emit():
                    pst = ps1.tile([HP, 1024], F32, tag="ps1", name="ps2")[:]
                    for ki in range(2):
                        k = k0 + ki
                        nrow = 2 if k == 9 else 4
                        for s in range(3):
                            rhs = yt_r[
                                :, 4 * k + s : 4 * k + s + nrow, :
                            ].rearrange("p d h -> p (d h)")
                            nc.tensor.matmul(
                                pst[:, 512 * ki : 512 * ki + 128 * nrow],
                                bw,
                                rhs,
                                start=(s == 0),
                                stop=(s == 2),
                            )
                    nrows = 6 if k0 == 8 else 8
                    dst = t3_r[:, 4 * k0 : 4 * k0 + nrows, :].rearrange(
                        "p r h -> p (r h)"
                    )
                    evac(dst, pst[:, 0 : 128 * nrows])
                return emit
            return [mk(k0) for k0 in range(0, 10, 2)]

        def d_final(v, t3_r, eng):
            B = bxp.tile([HP, N_BOX], F16, tag=f"box{v}", name=f"box{v}")[:]
            B_r = B.rearrange("p (do h) -> p do h", h=HP)
            eng.tensor_add(B_r, t3_r[:, 0:DQ, :], t3_r[:, 3 : 3 + DQ, :])
            eng.tensor_add(B_r, B_r, t3_r[:, 6 : 6 + DQ, :])
            return B

        def pass2_chunks9(v, yt_r):
            """9-shift pass2: box filter D-combination entirely on the PE;
            evacuates the final box directly (no t3 / d_final)."""
            bw = bandl if v in ("I", "J") else bandq
            B = bxp.tile([HP, N_BOX], F16, tag=f"box{v}", name=f"box{v}")[:]
            boxes[v] = B
            B_r = B.rearrange("p (do h) -> p do h", h=HP)
            def mk(g0):
                def emit():
                    pst = ps1.tile([HP, 1024], F32, tag="ps1", name="ps9")[:]
                    for gi in range(2):
                        g = g0 + gi
                        for s in range(9):
                            rhs = yt_r[
                                :, 4 * g + s : 4 * g + s + 4, :
                            ].rearrange("p d h -> p (d h)")
                            nc.tensor.matmul(
                                pst[:, 512 * gi : 512 * (gi + 1)],
                                bw,
                                rhs,
                                start=(s == 0),
                                stop=(s == 8),
                            )
                    dst = B_r[:, 4 * g0 : 4 * g0 + 8, :].rearrange(
                        "p r h -> p (r h)"
                    )
                    evac(dst, pst)
                return emit
            return [mk(g0) for g0 in (0, 2, 4, 6)]

        # ---- smoothness Gram chunks (PE) ----
        def g_chunk(c, r_lo, r_hi, with_end):
            def emit():
                fl = fl_ap[c]
                for r in range(r_lo, r_hi):
                    rhs = fl[:, r : r + 2, :].rearrange("p d w -> p (d w)")
                    nc.tensor.matmul(
                        psG[:, 0:256],
                        fl[:, r, :],
                        rhs,
                        start=(c == 0 and r == 0),
                        stop=False,
                        skip_group_check=True,
                    )
                if with_end:
                    nc.tensor.matmul(
                        psG[:, 0:128], fl[:, 32, :], fl[:, 32, :],
                        start=False, stop=False, skip_group_check=True,
                    )
                    nc.tensor.matmul(
                        psG[:, 256:384], fl[:, 0, :], fl[:, 0, :],
                        start=False, stop=False, skip_group_check=True,
                    )
                    nc.tensor.matmul(
                        psG[:, 384:512], fl[:, 32, :], fl[:, 32, :],
                        start=False, stop=(c == 2), skip_group_check=True,
                    )
            return emit

        def gp_chunk(c, r_lo, r_hi):
            def emit():
                ft = flt_ap[c]
                for r in range(r_lo, r_hi):
                    nc.tensor.matmul(
                        psGp,
                        ft[:, r, :],
                        ft[:, r, :],
                        start=(c == 0 and r == 0),
                        stop=(c == 2 and r == FLOWT_D - 1),
                        skip_group_check=True,
                    )
            return emit

        extra_q = []
        if not NO_GRAM:
            for c in range(3):
                extra_q.append(g_chunk(c, 0, 16, False))
                extra_q.append(g_chunk(c, 16, 32, True))
            for c in range(3):
                extra_q.append(gp_chunk(c, 0, 16))
                extra_q.append(gp_chunk(c, 16, 32))
        extra_i = [0]

        def pop_extra():
            if extra_i[0] < len(extra_q):
                extra_q[extra_i[0]]()
                extra_i[0] += 1

        # ---- mse (GPSIMD) ----
        inJmid = inJ_r[:, 4 : 4 + DQ, :].rearrange("p d w -> p (d w)")

        def mse():
            for h in range(2):
                lo, hi = 2048 * h, 2048 * (h + 1)
                md = scp.tile([HP, 2048], F16, tag="tP", name=f"md{h}")[:]
                md2 = scp.tile([HP, 2048], F16, tag="tQ", name=f"md2{h}")[:]
                eng = nc.vector if NO_GPS else nc.gpsimd
                eng.tensor_sub(md, inJmid[:, lo:hi], recon[:, lo:hi])
                nc.scalar.activation(
                    md2, md, AF.Square,
                    accum_out=acc[:, COL_MSE + h : COL_MSE + h + 1],
                )

        # ---- software-pipelined emission ----
        boxes = {}
        yt_rs, t3_rs = {}, {}

        def begin_var(v):
            ytt = ytp.tile([HP, N_YT], F16, tag="yt", name=f"yt{v}")[:]
            yt_rs[v] = ytt.rearrange("p (d h) -> p d h", h=HP)
            return pass1_chunks(v, srcs[v], yt_rs[v])

        def begin_pass2(v):
            t3t = t3p.tile([HP, N_T3], F16, tag="t3", name=f"t3{v}")[:]
            t3_rs[v] = t3t.rearrange("p (r h) -> p r h", h=HP)
            return pass2_chunks(v, yt_rs[v], t3_rs[v])

        # cc intermediates overwrite dead box tiles in place:
        #   after I:  m3 = bj^2 (GPS -> tmp3); m1 = bi*bj -> bxJ; m2 = bi^2 -> bxI
        #   after II: Ivar = bii - m2 -> bxI
        #   after JJ: Jvar = bjj - m3 -> bxJJ; den = Ivar*Jvar -> bxII;
        #             lnd = Ln(den+eps) -> bxJJ; rcp = Exp(-lnd) -> bxII
        #   tail(IJ): cross = bij - m1 -> bxJ; c2 = cross^2 -> bxIJ;
        #             cw = c2*rcp -> bxJ; ACT copy-accum
        tmp3 = pp.tile([HP, N_BOX], F16, tag="tmp3", name="tmp3")[:]

        def post_var(v):
            gps = nc.vector if NO_GPS else nc.gpsimd
            if v == "J":
                boxes["J"] = d_final("J", t3_rs["J"], gps)
            elif v == "I":
                boxes["I"] = d_final("I", t3_rs["I"], nc.vector)
            elif v == "II":
                boxes["II"] = d_final("II", t3_rs["II"], nc.vector)
                bj, bi = boxes["J"], boxes["I"]
                nc.scalar.activation(tmp3, bj, AF.Square)  # m3 = bj^2
                nc.vector.tensor_mul(bj, bj, bi)         # m1 = bi*bj -> bxJ
                nc.vector.tensor_mul(bi, bi, bi)         # m2 = bi^2  -> bxI
            elif v == "JJ":
                boxes["JJ"] = d_final("JJ", t3_rs["JJ"], nc.vector)
                nc.vector.tensor_sub(
                    boxes["I"], boxes["II"], boxes["I"]
                )                                         # Ivar -> bxI
                nc.vector.tensor_sub(boxes["JJ"], boxes["JJ"], tmp3)  # Jvar
                nc.vector.tensor_mul(
                    boxes["II"], boxes["I"], boxes["JJ"]
                )                                         # den -> bxII
                nc.scalar.activation(
                    boxes["JJ"], boxes["II"], AF.Ln, bias=eps_ap
                )                                         # lnd -> bxJJ
                nc.scalar.activation(
                    boxes["II"], boxes["JJ"], AF.Exp, scale=-1.0
                )                                         # rcp -> bxII

        pending_p2 = None
        pending_v = None
        prod_ahead = {"I": "II", "JJ": "IJ"}
        extras_on = {"II", "JJ", "IJ"}
        for v in VARS:
            p1 = begin_var(v)
            if pending_p2 is None:
                for e in p1:
                    e()
            else:
                ahead = prod_ahead.get(v)
                for ci in range(len(p1)):
                    if ci < len(pending_p2):
                        pending_p2[ci]()
                    p1[ci]()
                    if ahead and ci < 2:
                        product_chunk(ahead, ci)
                    if v in extras_on and ci >= 1:
                        pop_extra()
                post_var(pending_v)
            pending_p2 = begin_pass2(v)
            pending_v = v
        ij_p2 = pending_p2

        # last var (IJ): emit pass2 chunks with remaining gram work
        LASTV = VARS[-1]
        for e in ij_p2:
            e()
            pop_extra()
        while extra_i[0] < len(extra_q):
            pop_extra()

        # smoothness masked reductions
        if not NO_GRAM:
            if NO_TTR:
                nc.vector.tensor_copy(gs1, psG)
                nc.vector.tensor_mul(gs1, gs1, masks[:, 0:512])
                nc.scalar.activation(
                    gs1, gs1, AF.Copy,
                    accum_out=acc[:, COL_SMG : COL_SMG + 1],
                )
                nc.vector.tensor_copy(gs2, psGp)
                nc.vector.tensor_mul(gs2, gs2, masks[:, 512:640])
                nc.scalar.activation(
                    gs2, gs2, AF.Copy,
                    accum_out=acc[:, COL_SMGP : COL_SMGP + 1],
                )
            else:
                nc.vector.tensor_tensor_reduce(
                    out=gs1, in0=psG, in1=masks[:, 0:512],
                    scale=1.0, scalar=0.0, op0=ALU.mult, op1=ALU.add,
                    accum_out=acc[:, COL_SMG : COL_SMG + 1],
                )
                nc.vector.tensor_tensor_reduce(
                    out=gs2, in0=psGp, in1=masks[:, 512:640],
                    scale=1.0, scalar=0.0, op0=ALU.mult, op1=ALU.add,
                    accum_out=acc[:, COL_SMGP : COL_SMGP + 1],
                )

        mse()
        Blast = bxp.tile([HP, N_BOX], F16, tag=f"box{LASTV}", name=f"box{LASTV}")[:]
        Blast_r = Blast.rearrange("p (do h) -> p do h", h=HP)
        boxes[LASTV] = Blast

        NROW_SL = DQ // NSL

        def d_final_last_slice(sl):
            do0 = NROW_SL * sl
            t3r = t3_rs[LASTV]
            nc.vector.tensor_add(
                Blast_r[:, do0 : do0 + NROW_SL, :],
                t3r[:, do0 : do0 + NROW_SL, :],
                t3r[:, do0 + 3 : do0 + 3 + NROW_SL, :],
            )
            nc.vector.tensor_add(
                Blast_r[:, do0 : do0 + NROW_SL, :],
                Blast_r[:, do0 : do0 + NROW_SL, :],
                t3r[:, do0 + 6 : do0 + 6 + NROW_SL, :],
            )

        # cc tail per slice: cross = bij - m1; cc = cross^2 * rcp; accumulate
        def cc_slice(sl):
            lo, hi = sl * NS, (sl + 1) * NS
            m1 = boxes["J"][:, lo:hi]
            rcp = boxes["II"][:, lo:hi]
            bij = Blast[:, lo:hi]
            nc.vector.tensor_sub(m1, bij, m1)          # cross -> bxJ
            nc.vector.tensor_mul(bij, m1, m1)          # cross^2 -> bxIJ
            nc.vector.tensor_mul(m1, bij, rcp)         # cc -> bxJ
            nc.scalar.activation(
                m1, m1, AF.Copy,
                accum_out=acc[:, COL_CC + sl : COL_CC + sl + 1],
            )

        for sl in range(NSL):
            d_final_last_slice(sl)
            cc_slice(sl)

        nc.sync.dma_start(out=d_out, in_=acc)

    nc.compile()
    return nc


def _make_consts():
    k = np.arange(HP)
    band = (np.abs(k[:, None] - k[None, :]) <= 4).astype(np.float16)
    bandq = (band * np.float16(S16)).astype(np.float16)
    bandl = (band * np.float16(TLIN)).astype(np.float16)

    # quadratic difference mask: diag [1,2,...,2,1], off-diag -1
    dg = np.full(HP, 2.0)
    dg[0] = 1.0
    dg[-1] = 1.0
    Mq = np.diag(dg)
    Mq += np.diag(np.full(HP - 1, -1.0), 1)
    Mq += np.diag(np.full(HP - 1, -1.0), -1)
    I = np.eye(HP)
    masks = np.zeros((HP, N_MASK), np.float16)
    masks[:, 0:128] = Mq + 2 * I          # on G
    masks[:, 128:256] = -2 * I            # on C
    masks[:, 256:384] = -I                # on A
    masks[:, 384:512] = -(I + Mq)         # on B
    masks[:, 512:640] = Mq                # on G'
    return band, bandq, bandl, masks


def _shard_inputs(imgsA, recon_A, warped_BA, flow_BA):
    bandh, bandq, bandl, masks = _make_consts()
    in_maps = []
    for core in range(8):
        b, q = divmod(core, 4)
        d0 = DQ * q

        def slab(vol):
            s = np.zeros((HP, D_IN, W), np.float16)
            lo, hi = d0 - 4, d0 + DQ + 4
            clo, chi = max(lo, 0), min(hi, D_FULL)
            s[:, clo - lo : chi - lo, :] = vol[clo:chi].transpose(1, 0, 2)
            return s.reshape(HP, N_IN)

        rec = (
            recon_A[b, 0, d0 : d0 + DQ]
            .transpose(1, 0, 2)
            .astype(np.float16)
            .reshape(HP, N_RECON)
        )

        fl = np.empty((HP, 3, FLOW_D, W), np.float16)
        hi = min(d0 + FLOW_D, D_FULL)
        n = hi - d0
        fl[:, :, :n] = flow_BA[b, :, d0:hi].transpose(2, 0, 1, 3)
        if n < FLOW_D:
            fl[:, :, n:] = fl[:, :, n - 1 : n]

        # transposed flow tiles: [w, (c, d, h)]
        flt = (
            flow_BA[b, :, d0 : d0 + DQ]
            .transpose(3, 0, 1, 2)
            .astype(np.float16)
        )

        in_maps.append(
            {
                "inI": slab(warped_BA[b, 0]),
                "inJ": slab(imgsA[b, 0]),
                "recon": np.ascontiguousarray(rec),
                "flow": np.ascontiguousarray(fl).reshape(HP, 3 * N_FLOW_C),
                "flowT": np.ascontiguousarray(flt).reshape(HP, 3 * N_FLOWT_C),
                "bandh": bandh,
                "bandq": bandq,
                "bandl": bandl,
                "masks": masks,
            }
        )
    return in_maps


def _install_profile_shim():
    """Wire up NTFF profiling under axon when antenv.axon_hooks is absent."""
    try:
        import antenv.axon_hooks  # noqa: F401

        return True
    except ImportError:
        pass
    import contextlib
    import ctypes
    import types

    so_path = "/opt/axon/libaxon_pjrt.so"
    if not os.path.exists(so_path):
        return False
    lib = ctypes.CDLL(so_path)
    if not hasattr(lib, "axon_start_nrt_profile"):
        return False
    lib.axon_start_nrt_profile.argtypes = [
        ctypes.POINTER(ctypes.c_int64),
        ctypes.c_size_t,
    ]
    lib.axon_start_nrt_profile.restype = ctypes.c_int64
    lib.axon_stop_nrt_profile.argtypes = [ctypes.c_char_p]
    lib.axon_stop_nrt_profile.restype = ctypes.c_int64

    @contextlib.contextmanager
    def _hook(output_dir, device_ids):
        import jax

        jax.devices()
        if device_ids:
            ids = (ctypes.c_int64 * len(device_ids))(*device_ids)
            rc = lib.axon_start_nrt_profile(ids, len(device_ids))
        else:
            rc = lib.axon_start_nrt_profile(None, 0)
        if rc != 0:
            raise RuntimeError(f"axon_start_nrt_profile rc={rc}")
        try:
            yield
        finally:
            n = lib.axon_stop_nrt_profile(str(output_dir).encode())
            print(f"ntff profile: {n} file(s) written to {output_dir}")

    mod = types.ModuleType("antenv.axon_hooks")
    mod.get_axon_ntff_profile_hook = lambda: _hook
    mod.set_axon_ntff_profile_hook = lambda h: None
    import antenv

    sys.modules["antenv.axon_hooks"] = mod
    antenv.axon_hooks = mod

    import concourse.bass_utils as _bu

    _bu.upload_artifacts = lambda tmpdir: tmpdir
    return True


LAST_EXEC_NS = None
LAST_RESULTS = None


def kernel(imgsA, recon_A, warped_BA, flow_BA):
    global LAST_EXEC_NS, LAST_RESULTS
    if "nc" not in _CACHE:
        _CACHE["nc"] = _build_program()
    nc = _CACHE["nc"]

    in_maps = _shard_inputs(
        np.asarray(imgsA, np.float32),
        np.asarray(recon_A, np.float32),
        np.asarray(warped_BA, np.float32),
        np.asarray(flow_BA, np.float32),
    )
    trace = os.environ.get("GVSL_TRACE", "0") == "1"
    if trace:
        trace = _install_profile_shim()
    tmpdir = os.environ.get("GVSL_TRACE_DIR") or None
    res = run_bass_kernel_spmd(
        nc, in_maps, core_ids=list(range(8)), trace=trace, tmpdir=tmpdir
    )
    LAST_EXEC_NS = res.exec_time_ns
    LAST_RESULTS = res

    cc = mse_s = smg = smgp = 0.0
    for r in res.results:
        o = np.asarray(r["out"], np.float64)
        cc += o[:, COL_CC : COL_CC + NSL].sum()
        mse_s += o[:, COL_MSE : COL_MSE + 4].sum()
        smg += o[:, COL_SMG].sum()
        smgp += o[:, COL_SMGP].sum()

    n_vox = 2 * 1 * 128 * 128 * 128
    n_d = 2 * 3 * 127 * 128 * 128
    ncc_loss = 1.0 - cc / n_vox
    mse_loss = mse_s / n_vox
    smooth_loss = (smg + smgp) / (3.0 * n_d)
    return (
        np.float32(ncc_loss),
        np.float32(mse_loss),
        np.float32(smooth_loss),
    )


# revision 22
# speedup vs baseline: 1.0143x; 1.0143x over previous
"""GVSL loss (NCC + MSE + smoothness) as a distributed Bass kernel on 8 TRN2 cores.

Sharding: batch(2) x depth-quarters(4) = 8 shards; each core owns a 32-deep
output slab (+4-voxel halo for the 9^3 box filter).

NCC box filter strategy (per var in {I, J, I^2, J^2, IJ}):
  pass1 (PE):  per d-row matmul(lhsT=V_d[h,w], rhs=BandH[h,h']) -> PSUM [w, h]
               = H-box + transpose in one shot (fp16, FD=128)
  evac1:       PSUM -> SBUF fp16 YT [w, (d, h)]   (rotating DVE/ACT)
  pass2 (PE):  stationary BandW (scaled); 3 d-shifted FD=512 matmuls accumulate
               -> t3[r] = Z[r]+Z[r+1]+Z[r+2] (W-box + D-triple), PSUM
  evac2:       PSUM -> SBUF fp16 T3
  D-final:     S = t3[d] + t3[d+3] + t3[d+6]  (GPSIMD for var J, DVE otherwise)

cc math: cc = (crossS * rsqrt(IvarS*JvarS + eps))^2 -- Rsqrt+Square+Copy all
live in the reciprocal_sqrt_and_small ACT table set: one table load total.

Smoothness loss entirely on the PE via Gram matrices: for flow tiles
X_r = [h, w] per (channel, depth-row),
  sum dz^2 (w-diff)  = <Mq, G>        with G  = sum X_r^T X_r
  sum dy^2 (d-diff)  = <2I, G> - <I, A> - <I, B> - 2<I, C>,
                       C = sum X_r^T X_{r+1}, A = sum_c G_0, B = sum_c G_32
  sum dx^2 (h-diff)  = <Mq, G'>       with G' = sum X'_r^T X'_r (transposed)
All Gram matmuls accumulate into two PSUM banks; masked reduction is a single
fused DVE tensor_tensor_reduce per bank. Mq = quadratic diff mask
(diag [1,2,...,2,1], off-diag -1).

MSE: GPSIMD sub + fused scalar_tensor_tensor square-accumulate.
"""

import os
import sys

for _p in ("/opt/trn_rl_repo",):
    if _p not in sys.path:
        sys.path.insert(0, _p)

import numpy as np

import concourse.bass as bass
import concourse.tile as tile
from concourse import bacc, mybir
from concourse.bass_utils import run_bass_kernel_spmd

F32 = mybir.dt.float32
F16 = mybir.dt.float16
AF = mybir.ActivationFunctionType
ALU = mybir.AluOpType

HP = 128
W = 128
D_FULL = 128
DQ = 32
D_IN = DQ + 8     # 40 slab rows incl halo
YT_R = 40
T3_R = 40
FLOW_D = DQ + 1   # 33 rows for untransposed flow (d-pairs)
FLOWT_D = DQ      # 32 rows for transposed flow

N_IN = D_IN * W           # 5120
N_YT = YT_R * HP          # 5120
N_T3 = T3_R * HP          # 5120
N_BOX = DQ * HP           # 4096
N_RECON = DQ * W          # 4096
N_FLOW_C = FLOW_D * W     # 4224
N_FLOWT_C = FLOWT_D * HP  # 4096
N_MASK = 512 + 128        # 640

S16 = float(np.float16(1.0 / 27.0))          # quadratic-var scale
TLIN = float(np.float16(np.sqrt(S16 / 729.0)))  # linear-var scale
EPS_S = 1e-5 * S16 * S16

NSL = 4                   # cc slices
NS = N_BOX // NSL         # 1024 els per slice

COL_CC = 0     # +NSL
COL_MSE = COL_CC + NSL   # +4
COL_SMG = COL_MSE + 4
COL_SMGP = COL_SMG + 1
ACC_W = 12

VARS = ("J", "I", "II", "JJ", "IJ")

# evacuation engine rotation (PSUM -> SBUF copies): ACT-heavy
EVAC_PAT = ("scalar", "scalar", "vector")

_CACHE = {}

# bisect/debug knobs
NO_GRAM = os.environ.get("GVSL_NO_GRAM", "0") == "1"   # skip smooth Gram MMs + TTRs
NO_TTR = os.environ.get("GVSL_NO_TTR", "1") == "1"     # tensor_tensor_reduce hangs TRN2 HW; keep fallback
NO_GPS = os.environ.get("GVSL_NO_GPS", "0") == "1"     # no gpsimd work (all on DVE)


def _patch_act_tables():
    """Reorder activation-table sets so natural_log_exp_and_others (which
    contains ln + exp + square + copy) is preferred: one ACT table load."""
    from concourse import hw_specs

    if getattr(hw_specs, "_gvsl_patched", False):
        return
    orig = hw_specs.get_activation_tables

    def patched(arch):
        t = dict(orig(arch))
        key = "natural_log_exp_and_others"
        if key in t:
            t = {key: t[key], **{k: v for k, v in t.items() if k != key}}
        return t

    hw_specs.get_activation_tables = patched
    bacc.get_activation_tables = patched
    hw_specs._gvsl_patched = True


def _build_program():
    if os.environ.get("GVSL_PATCH_TABLES", "0") == "1":
        _patch_act_tables()
    nc = bacc.Bacc("TRN2", target_bir_lowering=False, debug=False, num_devices=8)

    d_inI = nc.dram_tensor("inI", [HP, N_IN], F16, kind="ExternalInput").ap()
    d_inJ = nc.dram_tensor("inJ", [HP, N_IN], F16, kind="ExternalInput").ap()
    d_recon = nc.dram_tensor("recon", [HP, N_RECON], F16, kind="ExternalInput").ap()
    d_flow = nc.dram_tensor("flow", [HP, 3 * N_FLOW_C], F16, kind="ExternalInput").ap()
    d_flowT = nc.dram_tensor(
        "flowT", [HP, 3 * N_FLOWT_C], F16, kind="ExternalInput"
    ).ap()
    d_bandh = nc.dram_tensor("bandh", [HP, HP], F16, kind="ExternalInput").ap()
    d_bandq = nc.dram_tensor("bandq", [HP, HP], F16, kind="ExternalInput").ap()
    d_bandl = nc.dram_tensor("bandl", [HP, HP], F16, kind="ExternalInput").ap()
    d_masks = nc.dram_tensor("masks", [HP, N_MASK], F16, kind="ExternalInput").ap()
    d_out = nc.dram_tensor("out", [HP, ACC_W], F32, kind="ExternalOutput").ap()

    from contextlib import ExitStack

    with tile.TileContext(nc) as tc, ExitStack() as es:
        pp = es.enter_context(tc.tile_pool(name="persist", bufs=1))
        prp = es.enter_context(tc.tile_pool(name="prodp", bufs=1))
        ytp = es.enter_context(tc.tile_pool(name="ytp", bufs=2))
        t3p = es.enter_context(tc.tile_pool(name="t3p", bufs=2))
        bxp = es.enter_context(tc.tile_pool(name="boxp", bufs=1))
        scp = es.enter_context(tc.tile_pool(name="scrp", bufs=1))
        flp = es.enter_context(tc.tile_pool(name="flowp", bufs=3))
        ps1 = es.enter_context(tc.tile_pool(name="psum1", bufs=3, space="PSUM"))
        psg = es.enter_context(tc.tile_pool(name="psumG", bufs=1, space="PSUM"))

        acc = pp.tile([HP, ACC_W], F32, tag="acc", name="acc")[:]
        nc.gpsimd.memset(acc, 0.0)
        eps_ap = pp.tile([HP, 1], F32, tag="epsc", name="epsc")[:]
        nc.gpsimd.memset(eps_ap, EPS_S)

        bandh = pp.tile([HP, HP], F16, tag="bandh", name="bandh")[:]
        bandq = pp.tile([HP, HP], F16, tag="bandq", name="bandq")[:]
        bandl = pp.tile([HP, HP], F16, tag="bandl", name="bandl")[:]
        masks = pp.tile([HP, N_MASK], F16, tag="masks", name="masks")[:]
        inI = pp.tile([HP, N_IN], F16, tag="inI", name="inI")[:]
        inJ = pp.tile([HP, N_IN], F16, tag="inJ", name="inJ")[:]
        recon = pp.tile([HP, N_RECON], F16, tag="recon", name="recon")[:]
        gs1 = pp.tile([HP, 512], F32, tag="gs1", name="gs1")[:]
        gs2 = pp.tile([HP, HP], F32, tag="gs2", name="gs2")[:]

        # PSUM Gram accumulators: [G | C | A | B] and G'
        psG = psg.tile([HP, 512], F32, tag="G", name="psG")[:]
        psGp_full = psg.tile([HP, 512], F32, tag="Gp", name="psGp")[:]
        psGp = psGp_full[:, 0:HP]

        NQ = N_IN // 4
        for q in range(4):
            nc.sync.dma_start(
                out=inJ[:, NQ * q : NQ * (q + 1)],
                in_=d_inJ[:, NQ * q : NQ * (q + 1)],
            )
        nc.sync.dma_start(out=bandh, in_=d_bandh)
        for q in range(4):
            nc.sync.dma_start(
                out=inI[:, NQ * q : NQ * (q + 1)],
                in_=d_inI[:, NQ * q : NQ * (q + 1)],
            )
        nc.sync.dma_start(out=bandl, in_=d_bandl)
        nc.sync.dma_start(out=bandq, in_=d_bandq)
        nc.sync.dma_start(out=masks, in_=d_masks)

        # flow channels stream in per-channel ring buffers
        fl_ap = {}
        flt_ap = {}
        for c in range(3):
            t = flp.tile([HP, N_FLOW_C], F16, tag="flc", name=f"flc{c}")[:]
            nc.sync.dma_start(
                out=t, in_=d_flow[:, c * N_FLOW_C : (c + 1) * N_FLOW_C]
            )
            fl_ap[c] = t.rearrange("p (d w) -> p d w", w=W)
        for c in range(3):
            t = flp.tile([HP, N_FLOWT_C], F16, tag="flt", name=f"flt{c}")[:]
            nc.sync.dma_start(
                out=t, in_=d_flowT[:, c * N_FLOWT_C : (c + 1) * N_FLOWT_C]
            )
            flt_ap[c] = t.rearrange("p (d h) -> p d h", h=HP)
        nc.sync.dma_start(out=recon, in_=d_recon)

        inI_r = inI.rearrange("p (d w) -> p d w", w=W)
        inJ_r = inJ.rearrange("p (d w) -> p d w", w=W)

        evac_ct = [0]

        def evac(dst, src):
            eng = EVAC_PAT[evac_ct[0] % len(EVAC_PAT)]
            if eng == "vector":
                nc.vector.tensor_copy(dst, src)
            else:
                nc.scalar.copy(dst, src)
            evac_ct[0] += 1

        # ---- products: JJ full on GPSIMD; IJ, II chunked on DVE ----
        prod_tiles = {}
        srcs = {"J": inJ_r, "I": inI_r}
        for v in ("JJ", "IJ", "II"):
            prod_tiles[v] = prp.tile(
                [HP, N_IN], F16, tag=f"prod{v}", name=f"prod{v}"
            )[:]
            srcs[v] = prod_tiles[v].rearrange("p (d w) -> p d w", w=W)
        if NO_GPS:
            nc.vector.tensor_mul(prod_tiles["JJ"], inJ, inJ)
        else:
            nc.gpsimd.tensor_mul(prod_tiles["JJ"], inJ, inJ)

        def product_chunk(v, c, n=2):
            lo = (N_IN // n) * c
            hi = (N_IN // n) * (c + 1)
            a = inJ if v == "JJ" else inI
            b = inJ if v in ("JJ", "IJ") else inI
            nc.vector.tensor_mul(prod_tiles[v][:, lo:hi], a[:, lo:hi], b[:, lo:hi])

        # ---- pass1 / pass2 ----
        def pass1_chunks(v, src_r, yt_r):
            def mk(g0):
                def emit():
                    pst = ps1.tile([HP, 1024], F32, tag="ps1", name="ps1")[:]
                    for q in range(8):
                        nc.tensor.matmul(
                            pst[:, 128 * q : 128 * (q + 1)],
                            src_r[:, g0 + q, :],
                            bandh,
                            start=True,
                            stop=True,
                        )
                    dst = yt_r[:, g0 : g0 + 8, :].rearrange("p d h -> p (d h)")
                    evac(dst, pst)
                return emit
            return [mk(g0) for g0 in range(0, D_IN, 8)]

        def pass2_chunks(v, yt_r, t3_r):
            bw = bandl if v in ("I", "J") else bandq
            def mk(k0):
                def emit():
                    pst = ps1.tile([HP, 1024], F32, tag="ps1", name="ps2")[:]
                    for ki in range(2):
                        k = k0 + ki
                        nrow = 2 if k == 9 else 4
                        for s in range(3):
                            rhs = yt_r[
                                :, 4 * k + s : 4 * k + s + nrow, :
                            ].rearrange("p d h -> p (d h)")
                            nc.tensor.matmul(
                                pst[:, 512 * ki : 512 * ki + 128 * nrow],
                                bw,
                                rhs,
                                start=(s == 0),
                                stop=(s == 2),
                            )
                    nrows = 6 if k0 == 8 else 8
                    dst = t3_r[:, 4 * k0 : 4 * k0 + nrows, :].rearrange(
                        "p r h -> p (r h)"
                    )
                    evac(dst, pst[:, 0 : 128 * nrows])
                return emit
            return [mk(k0) for k0 in range(0, 10, 2)]

        def d_final(v, t3_r, eng):
            B = bxp.tile([HP, N_BOX], F16, tag=f"box{v}", name=f"box{v}")[:]
            B_r = B.rearrange("p (do h) -> p do h", h=HP)
            eng.tensor_add(B_r, t3_r[:, 0:DQ, :], t3_r[:, 3 : 3 + DQ, :])
            eng.tensor_add(B_r, B_r, t3_r[:, 6 : 6 + DQ, :])
            return B

        def pass2_chunks9(v, yt_r):
            """9-shift pass2: box filter D-combination entirely on the PE;
            evacuates the final box directly (no t3 / d_final)."""
            bw = bandl if v in ("I", "J") else bandq
            B = bxp.tile([HP, N_BOX], F16, tag=f"box{v}", name=f"box{v}")[:]
            boxes[v] = B
            B_r = B.rearrange("p (do h) -> p do h", h=HP)
            def mk(g0):
                def emit():
                    pst = ps1.tile([HP, 1024], F32, tag="ps1", name="ps9")[:]
                    for gi in range(2):
                        g = g0 + gi
                        for s in range(9):
                            rhs = yt_r[
                                :, 4 * g + s : 4 * g + s + 4, :
                            ].rearrange("p d h -> p (d h)")
                            nc.tensor.matmul(
                                pst[:, 512 * gi : 512 * (gi + 1)],
                                bw,
                                rhs,
                                start=(s == 0),
                                stop=(s == 8),
                            )
                    dst = B_r[:, 4 * g0 : 4 * g0 + 8, :].rearrange(
                        "p r h -> p (r h)"
                    )
                    evac(dst, pst)
                return emit
            return [mk(g0) for g0 in (0, 2, 4, 6)]

        # ---- smoothness Gram chunks (PE) ----
        def g_chunk(c, r_lo, r_hi, with_end):
            def emit():
                fl = fl_ap[c]
                for r in range(r_lo, r_hi):
                    rhs = fl[:, r : r + 2, :].rearrange("p d w -> p (d w)")
                    nc.tensor.matmul(
                        psG[:, 0:256],
                        fl[:, r, :],
                        rhs,
                        start=(c == 0 and r == 0),
                        stop=False,
                        skip_group_check=True,
                    )
                if with_end:
                    nc.tensor.matmul(
                        psG[:, 0:128], fl[:, 32, :], fl[:, 32, :],
                        start=False, stop=False, skip_group_check=True,
                    )
                    nc.tensor.matmul(
                        psG[:, 256:384], fl[:, 0, :], fl[:, 0, :],
                        start=False, stop=False, skip_group_check=True,
                    )
                    nc.tensor.matmul(
                        psG[:, 384:512], fl[:, 32, :], fl[:, 32, :],
                        start=False, stop=(c == 2), skip_group_check=True,
                    )
            return emit

        def gp_chunk(c, r_lo, r_hi):
            def emit():
                ft = flt_ap[c]
                for r in range(r_lo, r_hi):
                    nc.tensor.matmul(
                        psGp,
                        ft[:, r, :],
                        ft[:, r, :],
                        start=(c == 0 and r == 0),
                        stop=(c == 2 and r == FLOWT_D - 1),
                        skip_group_check=True,
                    )
            return emit

        extra_q = []
        if not NO_GRAM:
            for c in range(3):
                extra_q.append(g_chunk(c, 0, 16, False))
                extra_q.append(g_chunk(c, 16, 32, True))
            for c in range(3):
                extra_q.append(gp_chunk(c, 0, 16))
                extra_q.append(gp_chunk(c, 16, 32))
        extra_i = [0]

        def pop_extra():
            if extra_i[0] < len(extra_q):
                extra_q[extra_i[0]]()
                extra_i[0] += 1

        # ---- mse (GPSIMD) ----
        inJmid = inJ_r[:, 4 : 4 + DQ, :].rearrange("p d w -> p (d w)")

        def mse():
            for h in range(2):
                lo, hi = 2048 * h, 2048 * (h + 1)
                md = scp.tile([HP, 2048], F16, tag="tP", name=f"mse{h}")[:]
                md2 = scp.tile([HP, 2048], F16, tag="tQ", name=f"mse2{h}")[:]
                eng = nc.vector if NO_GPS else nc.gpsimd
                eng.tensor_sub(md, inJmid[:, lo:hi], recon[:, lo:hi])
                nc.scalar.activation(
                    md2, md, AF.Square,
                    accum_out=acc[:, COL_MSE + h : COL_MSE + h + 1],
                )

        # ---- software-pipelined emission ----
        boxes = {}
        yt_rs, t3_rs = {}, {}

        def begin_var(v):
            ytt = ytp.tile([HP, N_YT], F16, tag="yt", name=f"yt{v}")[:]
            yt_rs[v] = ytt.rearrange("p (d h) -> p d h", h=HP)
            return pass1_chunks(v, srcs[v], yt_rs[v])

        def begin_pass2(v):
            t3t = t3p.tile([HP, N_T3], F16, tag="t3", name=f"t3{v}")[:]
            t3_rs[v] = t3t.rearrange("p (r h) -> p r h", h=HP)
            return pass2_chunks(v, yt_rs[v], t3_rs[v])

        # cc intermediates overwrite dead box tiles in place:
        #   after I:  m3 = bj^2 (GPS -> tmp3); m1 = bi*bj -> bxJ; m2 = bi^2 -> bxI
        #   after II: Ivar = bii - m2 -> bxI
        #   after JJ: Jvar = bjj - m3 -> bxJJ; den = Ivar*Jvar -> bxII;
        #             lnd = Ln(den+eps) -> bxJJ; rcp = Exp(-lnd) -> bxII
        #   tail(IJ): cross = bij - m1 -> bxJ; c2 = cross^2 -> bxIJ;
        #             cw = c2*rcp -> bxJ; ACT copy-accum
        tmp3 = pp.tile([HP, N_BOX], F16, tag="tmp3", name="tmp3")[:]

        def post_var(v):
            gps = nc.vector if NO_GPS else nc.gpsimd
            if v == "J":
                boxes["J"] = d_final("J", t3_rs["J"], gps)
            elif v == "I":
                boxes["I"] = d_final("I", t3_rs["I"], nc.vector)
            elif v == "II":
                boxes["II"] = d_final("II", t3_rs["II"], nc.vector)
                bj, bi = boxes["J"], boxes["I"]
                nc.scalar.activation(tmp3, bj, AF.Square)  # m3 = bj^2
                nc.vector.tensor_mul(bj, bj, bi)         # m1 = bi*bj -> bxJ
                nc.vector.tensor_mul(bi, bi, bi)         # m2 = bi^2  -> bxI
            elif v == "JJ":
                boxes["JJ"] = d_final("JJ", t3_rs["JJ"], nc.vector)
                nc.vector.tensor_sub(
                    boxes["I"], boxes["II"], boxes["I"]
                )                                         # Ivar -> bxI
                nc.vector.tensor_sub(boxes["JJ"], boxes["JJ"], tmp3)  # Jvar
                nc.vector.tensor_mul(
                    boxes["II"], boxes["I"], boxes["JJ"]
                )                                         # den -> bxII
                nc.scalar.activation(
                    boxes["JJ"], boxes["II"], AF.Ln, bias=eps_ap
                )                                         # lnd -> bxJJ
                nc.scalar.activation(
                    boxes["II"], boxes["JJ"], AF.Exp, scale=-1.0
                )                                         # rcp -> bxII
                mse()

        pending_p2 = None
        pending_v = None
        prod_ahead = {"I": "II", "JJ": "IJ"}
        extras_on = {"II", "JJ", "IJ"}
        for v in VARS:
            p1 = begin_var(v)
            if pending_p2 is None:
                for e in p1:
                    e()
            else:
                ahead = prod_ahead.get(v)
                for ci in range(len(p1)):
                    if ci < len(pending_p2):
                        pending_p2[ci]()
                    p1[ci]()
                    if ahead and ci < 2:
                        product_chunk(ahead, ci)
                    if v in extras_on and ci >= 1:
                        pop_extra()
                post_var(pending_v)
            pending_p2 = begin_pass2(v)
            pending_v = v
        ij_p2 = pending_p2

        # last var (IJ): emit pass2 chunks with remaining gram work
        LASTV = VARS[-1]
        for e in ij_p2:
            e()
            pop_extra()
        while extra_i[0] < len(extra_q):
            pop_extra()

        # smoothness masked reductions
        if not NO_GRAM:
            if NO_TTR:
                nc.vector.tensor_copy(gs1, psG)
                nc.vector.tensor_mul(gs1, gs1, masks[:, 0:512])
                nc.scalar.activation(
                    gs1, gs1, AF.Copy,
                    accum_out=acc[:, COL_SMG : COL_SMG + 1],
                )
                nc.vector.tensor_copy(gs2, psGp)
                nc.vector.tensor_mul(gs2, gs2, masks[:, 512:640])
                nc.scalar.activation(
                    gs2, gs2, AF.Copy,
                    accum_out=acc[:, COL_SMGP : COL_SMGP + 1],
                )
            else:
                nc.vector.tensor_tensor_reduce(
                    out=gs1, in0=psG, in1=masks[:, 0:512],
                    scale=1.0, scalar=0.0, op0=ALU.mult, op1=ALU.add,
                    accum_out=acc[:, COL_SMG : COL_SMG + 1],
                )
                nc.vector.tensor_tensor_reduce(
                    out=gs2, in0=psGp, in1=masks[:, 512:640],
                    scale=1.0, scalar=0.0, op0=ALU.mult, op1=ALU.add,
                    accum_out=acc[:, COL_SMGP : COL_SMGP + 1],
                )

        Blast = bxp.tile([HP, N_BOX], F16, tag=f"box{LASTV}", name=f"box{LASTV}")[:]
        Blast_r = Blast.rearrange("p (do h) -> p do h", h=HP)
        boxes[LASTV] = Blast

        NROW_SL = DQ // NSL

        def d_final_last_slice(sl):
            do0 = NROW_SL * sl
            t3r = t3_rs[LASTV]
            nc.vector.tensor_add(
                Blast_r[:, do0 : do0 + NROW_SL, :],
                t3r[:, do0 : do0 + NROW_SL, :],
                t3r[:, do0 + 3 : do0 + 3 + NROW_SL, :],
            )
            nc.vector.tensor_add(
                Blast_r[:, do0 : do0 + NROW_SL, :],
                Blast_r[:, do0 : do0 + NROW_SL, :],
                t3r[:, do0 + 6 : do0 + 6 + NROW_SL, :],
            )

        # cc tail per slice: cross = bij - m1; cc = cross^2 * rcp; accumulate
        def cc_slice(sl):
            lo, hi = sl * NS, (sl + 1) * NS
            m1 = boxes["J"][:, lo:hi]
            rcp = boxes["II"][:, lo:hi]
            bij = Blast[:, lo:hi]
            nc.vector.tensor_sub(m1, bij, m1)          # cross -> bxJ
            nc.vector.tensor_mul(bij, m1, m1)          # cross^2 -> bxIJ
            nc.vector.tensor_mul(m1, bij, rcp)         # cc -> bxJ
            nc.scalar.activation(
                m1, m1, AF.Copy,
                accum_out=acc[:, COL_CC + sl : COL_CC + sl + 1],
            )

        for sl in range(NSL):
            d_final_last_slice(sl)
            cc_slice(sl)

        nc.sync.dma_start(out=d_out, in_=acc)

    nc.compile()
    return nc


def _make_consts():
    k = np.arange(HP)
    band = (np.abs(k[:, None] - k[None, :]) <= 4).astype(np.float16)
    bandq = (band * np.float16(S16)).astype(np.float16)
    bandl = (band * np.float16(TLIN)).astype(np.float16)

    # quadratic difference mask: diag [1,2,...,2,1], off-diag -1
    dg = np.full(HP, 2.0)
    dg[0] = 1.0
    dg[-1] = 1.0
    Mq = np.diag(dg)
    Mq += np.diag(np.full(HP - 1, -1.0), 1)
    Mq += np.diag(np.full(HP - 1, -1.0), -1)
    I = np.eye(HP)
    masks = np.zeros((HP, N_MASK), np.float16)
    masks[:, 0:128] = Mq + 2 * I          # on G
    masks[:, 128:256] = -2 * I            # on C
    masks[:, 256:384] = -I                # on A
    masks[:, 384:512] = -(I + Mq)         # on B
    masks[:, 512:640] = Mq                # on G'
    return band, bandq, bandl, masks


def _shard_inputs(imgsA, recon_A, warped_BA, flow_BA):
    bandh, bandq, bandl, masks = _make_consts()
    in_maps = []
    for core in range(8):
        b, q = divmod(core, 4)
        d0 = DQ * q

        def slab(vol):
            s = np.zeros((HP, D_IN, W), np.float16)
            lo, hi = d0 - 4, d0 + DQ + 4
            clo, chi = max(lo, 0), min(hi, D_FULL)
            s[:, clo - lo : chi - lo, :] = vol[clo:chi].transpose(1, 0, 2)
            return s.reshape(HP, N_IN)

        rec = (
            recon_A[b, 0, d0 : d0 + DQ]
            .transpose(1, 0, 2)
            .astype(np.float16)
            .reshape(HP, N_RECON)
        )

        fl = np.empty((HP, 3, FLOW_D, W), np.float16)
        hi = min(d0 + FLOW_D, D_FULL)
        n = hi - d0
        fl[:, :, :n] = flow_BA[b, :, d0:hi].transpose(2, 0, 1, 3)
        if n < FLOW_D:
            fl[:, :, n:] = fl[:, :, n - 1 : n]

        # transposed flow tiles: [w, (c, d, h)]
        flt = (
            flow_BA[b, :, d0 : d0 + DQ]
            .transpose(3, 0, 1, 2)
            .astype(np.float16)
        )

        in_maps.append(
            {
                "inI": slab(warped_BA[b, 0]),
                "inJ": slab(imgsA[b, 0]),
                "recon": np.ascontiguousarray(rec),
                "flow": np.ascontiguousarray(fl).reshape(HP, 3 * N_FLOW_C),
                "flowT": np.ascontiguousarray(flt).reshape(HP, 3 * N_FLOWT_C),
                "bandh": bandh,
                "bandq": bandq,
                "bandl": bandl,
                "masks": masks,
            }
        )
    return in_maps


def _install_profile_shim():
    """Wire up NTFF profiling under axon when antenv.axon_hooks is absent."""
    try:
        import antenv.axon_hooks  # noqa: F401

        return True
    except ImportError:
        pass
    import contextlib
    import ctypes
    import types

    so_path = "/opt/axon/libaxon_pjrt.so"
    if not os.path.exists(so_path):
        return False
    lib = ctypes.CDLL(so_path)
    if not hasattr(lib, "axon_start_nrt_profile"):
        return False
    lib.axon_start_nrt_profile.argtypes = [
        ctypes.POINTER(ctypes.c_int64),
        ctypes.c_size_t,
    ]
    lib.axon_start_nrt_profile.restype = ctypes.c_int64
    lib.axon_stop_nrt_profile.argtypes = [ctypes.c_char_p]
    lib.axon_stop_nrt_profile.restype = ctypes.c_int64

    @contextlib.contextmanager
    def _hook(output_dir, device_ids):
        import jax

        jax.devices()
        if device_ids:
            ids = (ctypes.c_int64 * len(device_ids))(*device_ids)
            rc = lib.axon_start_nrt_profile(ids, len(device_ids))
        else:
            rc = lib.axon_start_nrt_profile(None, 0)
        if rc != 0:
            raise RuntimeError(f"axon_start_nrt_profile rc={rc}")
        try:
            yield
        finally:
            n = lib.axon_stop_nrt_profile(str(output_dir).encode())
            print(f"ntff profile: {n} file(s) written to {output_dir}")

    mod = types.ModuleType("antenv.axon_hooks")
    mod.get_axon_ntff_profile_hook = lambda: _hook
    mod.set_axon_ntff_profile_hook = lambda h: None
    import antenv

    sys.modules["antenv.axon_hooks"] = mod
    antenv.axon_hooks = mod

    import concourse.bass_utils as _bu

    _bu.upload_artifacts = lambda tmpdir: tmpdir
    return True


LAST_EXEC_NS = None
LAST_RESULTS = None


def kernel(imgsA, recon_A, warped_BA, flow_BA):
    global LAST_EXEC_NS, LAST_RESULTS
    if "nc" not in _CACHE:
        _CACHE["nc"] = _build_program()
    nc = _CACHE["nc"]

    in_maps = _shard_inputs(
        np.asarray(imgsA, np.float32),
        np.asarray(recon_A, np.float32),
        np.asarray(warped_BA, np.float32),
        np.asarray(flow_BA, np.float32),
    )
    trace = os.environ.get("GVSL_TRACE", "0") == "1"
    if trace:
        trace = _install_profile_shim()
    tmpdir = os.environ.get("GVSL_TRACE_DIR") or None
    res = run_bass_kernel_spmd(
        nc, in_maps, core_ids=list(range(8)), trace=trace, tmpdir=tmpdir
    )
    LAST_EXEC_NS = res.exec_time_ns
    LAST_RESULTS = res

    cc = mse_s = smg = smgp = 0.0
    for r in res.results:
        o = np.asarray(r["out"], np.float64)
        cc += o[:, COL_CC : COL_CC + NSL].sum()
        mse_s += o[:, COL_MSE : COL_MSE + 4].sum()
        smg += o[:, COL_SMG].sum()
        smgp += o[:, COL_SMGP].sum()

    n_vox = 2 * 1 * 128 * 128 * 128
    n_d = 2 * 3 * 127 * 128 * 128
    ncc_loss = 1.0 - cc / n_vox
    mse_loss = mse_s / n_vox
    smooth_loss = (smg + smgp) / (3.0 * n_d)
    return (
        np.float32(ncc_loss),
        np.float32(mse_loss),
        np.float32(smooth_loss),
    )


# revision 24
# speedup vs baseline: 1.0643x; 1.0493x over previous
"""GVSL loss (NCC + MSE + smoothness) as a distributed Bass kernel on 8 TRN2 cores.

Sharding: batch(2) x depth-quarters(4) = 8 shards; each core owns a 32-deep
output slab (+4-voxel halo for the 9^3 box filter).

NCC box filter strategy (per var in {I, J, I^2, J^2, IJ}):
  pass1 (PE):  per d-row matmul(lhsT=V_d[h,w], rhs=BandH[h,h']) -> PSUM [w, h]
               = H-box + transpose in one shot (fp16, FD=128)
  evac1:       PSUM -> SBUF fp16 YT [w, (d, h)]   (rotating DVE/ACT)
  pass2 (PE):  stationary BandW (scaled); 3 d-shifted FD=512 matmuls accumulate
               -> t3[r] = Z[r]+Z[r+1]+Z[r+2] (W-box + D-triple), PSUM
  evac2:       PSUM -> SBUF fp16 T3
  D-final:     S = t3[d] + t3[d+3] + t3[d+6]  (GPSIMD for var J, DVE otherwise)

cc math: cc = (crossS * rsqrt(IvarS*JvarS + eps))^2 -- Rsqrt+Square+Copy all
live in the reciprocal_sqrt_and_small ACT table set: one table load total.

Smoothness loss entirely on the PE via Gram matrices: for flow tiles
X_r = [h, w] per (channel, depth-row),
  sum dz^2 (w-diff)  = <Mq, G>        with G  = sum X_r^T X_r
  sum dy^2 (d-diff)  = <2I, G> - <I, A> - <I, B> - 2<I, C>,
                       C = sum X_r^T X_{r+1}, A = sum_c G_0, B = sum_c G_32
  sum dx^2 (h-diff)  = <Mq, G'>       with G' = sum X'_r^T X'_r (transposed)
All Gram matmuls accumulate into two PSUM banks; masked reduction is a single
fused DVE tensor_tensor_reduce per bank. Mq = quadratic diff mask
(diag [1,2,...,2,1], off-diag -1).

MSE: GPSIMD sub + fused scalar_tensor_tensor square-accumulate.
"""

import os
import sys

for _p in ("/opt/trn_rl_repo",):
    if _p not in sys.path:
        sys.path.insert(0, _p)

import numpy as np

import concourse.bass as bass
import concourse.tile as tile
from concourse import bacc, mybir
from concourse.bass_utils import run_bass_kernel_spmd

F32 = mybir.dt.float32
F16 = mybir.dt.float16
AF = mybir.ActivationFunctionType
ALU = mybir.AluOpType

HP = 128
W = 128
D_FULL = 128
DQ = 32
D_IN = DQ + 8     # 40 slab rows incl halo
YT_R = 40
T3_R = 40
FLOW_D = DQ + 1   # 33 rows for untransposed flow (d-pairs)
FLOWT_D = DQ      # 32 rows for transposed flow

N_IN = D_IN * W           # 5120
N_YT = YT_R * HP          # 5120
N_T3 = T3_R * HP          # 5120
N_BOX = DQ * HP           # 4096
N_RECON = DQ * W          # 4096
N_FLOW_C = FLOW_D * W     # 4224
N_FLOWT_C = FLOWT_D * HP  # 4096
N_MASK = 512 + 128        # 640

S16 = float(np.float16(1.0 / 27.0))          # quadratic-var scale
TLIN = float(np.float16(np.sqrt(S16 / 729.0)))  # linear-var scale
EPS_S = 1e-5 * S16 * S16

NSL = 4                   # cc slices
NS = N_BOX // NSL         # 1024 els per slice

COL_CC = 0     # +NSL
COL_MSE = COL_CC + NSL   # +4
COL_SMG = COL_MSE + 4
COL_SMGP = COL_SMG + 1
ACC_W = 12

VARS = ("J", "I", "II", "JJ", "IJ")

# evacuation engine rotation (PSUM -> SBUF copies): ACT-heavy
EVAC_PAT = ("scalar", "scalar", "scalar", "vector")

_CACHE = {}

# bisect/debug knobs
NO_GRAM = os.environ.get("GVSL_NO_GRAM", "0") == "1"   # skip smooth Gram MMs + TTRs
NO_TTR = os.environ.get("GVSL_NO_TTR", "1") == "1"     # tensor_tensor_reduce hangs TRN2 HW; keep fallback
NO_GPS = os.environ.get("GVSL_NO_GPS", "0") == "1"     # no gpsimd work (all on DVE)


def _patch_act_tables():
    """Reorder activation-table sets so natural_log_exp_and_others (which
    contains ln + exp + square + copy) is preferred: one ACT table load."""
    from concourse import hw_specs

    if getattr(hw_specs, "_gvsl_patched", False):
        return
    orig = hw_specs.get_activation_tables

    def patched(arch):
        t = dict(orig(arch))
        key = "natural_log_exp_and_others"
        if key in t:
            t = {key: t[key], **{k: v for k, v in t.items() if k != key}}
        return t

    hw_specs.get_activation_tables = patched
    bacc.get_activation_tables = patched
    hw_specs._gvsl_patched = True


def _build_program():
    if os.environ.get("GVSL_PATCH_TABLES", "1") == "1":
        _patch_act_tables()
    nc = bacc.Bacc("TRN2", target_bir_lowering=False, debug=False, num_devices=8)

    d_inI = nc.dram_tensor("inI", [HP, N_IN], F16, kind="ExternalInput").ap()
    d_inJ = nc.dram_tensor("inJ", [HP, N_IN], F16, kind="ExternalInput").ap()
    d_recon = nc.dram_tensor("recon", [HP, N_RECON], F16, kind="ExternalInput").ap()
    d_flow = nc.dram_tensor("flow", [HP, 3 * N_FLOW_C], F16, kind="ExternalInput").ap()
    d_flowT = nc.dram_tensor(
        "flowT", [HP, 3 * N_FLOWT_C], F16, kind="ExternalInput"
    ).ap()
    d_bandh = nc.dram_tensor("bandh", [HP, HP], F16, kind="ExternalInput").ap()
    d_bandq = nc.dram_tensor("bandq", [HP, HP], F16, kind="ExternalInput").ap()
    d_bandl = nc.dram_tensor("bandl", [HP, HP], F16, kind="ExternalInput").ap()
    d_masks = nc.dram_tensor("masks", [HP, N_MASK], F16, kind="ExternalInput").ap()
    d_out = nc.dram_tensor("out", [HP, ACC_W], F32, kind="ExternalOutput").ap()

    from contextlib import ExitStack

    with tile.TileContext(nc) as tc, ExitStack() as es:
        pp = es.enter_context(tc.tile_pool(name="persist", bufs=1))
        prp = es.enter_context(tc.tile_pool(name="prodp", bufs=1))
        ytp = es.enter_context(tc.tile_pool(name="ytp", bufs=2))
        t3p = es.enter_context(tc.tile_pool(name="t3p", bufs=2))
        bxp = es.enter_context(tc.tile_pool(name="boxp", bufs=1))
        scp = es.enter_context(tc.tile_pool(name="scrp", bufs=1))
        flp = es.enter_context(tc.tile_pool(name="flowp", bufs=3))
        ps1 = es.enter_context(tc.tile_pool(name="psum1", bufs=3, space="PSUM"))
        psg = es.enter_context(tc.tile_pool(name="psumG", bufs=1, space="PSUM"))

        acc = pp.tile([HP, ACC_W], F32, tag="acc", name="acc")[:]
        nc.gpsimd.memset(acc, 0.0)
        eps_ap = pp.tile([HP, 1], F32, tag="epsc", name="epsc")[:]
        nc.gpsimd.memset(eps_ap, EPS_S)

        bandh = pp.tile([HP, HP], F16, tag="bandh", name="bandh")[:]
        bandq = pp.tile([HP, HP], F16, tag="bandq", name="bandq")[:]
        bandl = pp.tile([HP, HP], F16, tag="bandl", name="bandl")[:]
        masks = pp.tile([HP, N_MASK], F16, tag="masks", name="masks")[:]
        inI = pp.tile([HP, N_IN], F16, tag="inI", name="inI")[:]
        inJ = pp.tile([HP, N_IN], F16, tag="inJ", name="inJ")[:]
        recon = pp.tile([HP, N_RECON], F16, tag="recon", name="recon")[:]
        gs1 = pp.tile([HP, 512], F32, tag="gs1", name="gs1")[:]
        gs2 = pp.tile([HP, HP], F32, tag="gs2", name="gs2")[:]

        # PSUM Gram accumulators: [G | C | A | B] and G'
        psG = psg.tile([HP, 512], F32, tag="G", name="psG")[:]
        psGp_full = psg.tile([HP, 512], F32, tag="Gp", name="psGp")[:]
        psGp = psGp_full[:, 0:HP]

        NQ = N_IN // 4
        for q in range(4):
            nc.sync.dma_start(
                out=inJ[:, NQ * q : NQ * (q + 1)],
                in_=d_inJ[:, NQ * q : NQ * (q + 1)],
            )
        nc.sync.dma_start(out=bandh, in_=d_bandh)
        for q in range(4):
            nc.sync.dma_start(
                out=inI[:, NQ * q : NQ * (q + 1)],
                in_=d_inI[:, NQ * q : NQ * (q + 1)],
            )
        nc.sync.dma_start(out=bandl, in_=d_bandl)
        nc.sync.dma_start(out=bandq, in_=d_bandq)
        nc.sync.dma_start(out=masks, in_=d_masks)

        # flow channels stream in per-channel ring buffers
        fl_ap = {}
        flt_ap = {}
        for c in range(3):
            t = flp.tile([HP, N_FLOW_C], F16, tag="flc", name=f"flc{c}")[:]
            nc.sync.dma_start(
                out=t, in_=d_flow[:, c * N_FLOW_C : (c + 1) * N_FLOW_C]
            )
            fl_ap[c] = t.rearrange("p (d w) -> p d w", w=W)
        for c in range(3):
            t = flp.tile([HP, N_FLOWT_C], F16, tag="flt", name=f"flt{c}")[:]
            nc.sync.dma_start(
                out=t, in_=d_flowT[:, c * N_FLOWT_C : (c + 1) * N_FLOWT_C]
            )
            flt_ap[c] = t.rearrange("p (d h) -> p d h", h=HP)
        nc.sync.dma_start(out=recon, in_=d_recon)

        inI_r = inI.rearrange("p (d w) -> p d w", w=W)
        inJ_r = inJ.rearrange("p (d w) -> p d w", w=W)

        evac_ct = [0]

        def evac(dst, src):
            eng = EVAC_PAT[evac_ct[0] % len(EVAC_PAT)]
            if eng == "vector":
                nc.vector.tensor_copy(dst, src)
            else:
                nc.scalar.copy(dst, src)
            evac_ct[0] += 1

        # ---- products: JJ full on GPSIMD; IJ, II chunked on DVE ----
        prod_tiles = {}
        srcs = {"J": inJ_r, "I": inI_r}
        for v in ("JJ", "IJ", "II"):
            prod_tiles[v] = prp.tile(
                [HP, N_IN], F16, tag=f"prod{v}", name=f"prod{v}"
            )[:]
            srcs[v] = prod_tiles[v].rearrange("p (d w) -> p d w", w=W)
        peng = nc.vector if NO_GPS else nc.gpsimd
        peng.tensor_mul(prod_tiles["II"], inI, inI)
        peng.tensor_mul(prod_tiles["JJ"], inJ, inJ)

        def product_chunk(v, c, n=2):
            lo = (N_IN // n) * c
            hi = (N_IN // n) * (c + 1)
            a = inJ if v == "JJ" else inI
            b = inJ if v in ("JJ", "IJ") else inI
            nc.vector.tensor_mul(prod_tiles[v][:, lo:hi], a[:, lo:hi], b[:, lo:hi])

        # ---- pass1 / pass2 ----
        def pass1_chunks(v, src_r, yt_r):
            def mk(g0):
                def emit():
                    pst = ps1.tile([HP, 1024], F32, tag="ps1", name="ps1")[:]
                    for q in range(8):
                        nc.tensor.matmul(
                            pst[:, 128 * q : 128 * (q + 1)],
                            src_r[:, g0 + q, :],
                            bandh,
                            start=True,
                            stop=True,
                        )
                    dst = yt_r[:, g0 : g0 + 8, :].rearrange("p d h -> p (d h)")
                    evac(dst, pst)
                return emit
            return [mk(g0) for g0 in range(0, D_IN, 8)]

        def pass2_chunks(v, yt_r, t3_r):
            bw = bandl if v in ("I", "J") else bandq
            def mk(k0):
                def emit():
                    pst = ps1.tile([HP, 1024], F32, tag="ps1", name="ps2")[:]
                    for ki in range(2):
                        k = k0 + ki
                        nrow = 2 if k == 9 else 4
                        for s in range(3):
                            rhs = yt_r[
                                :, 4 * k + s : 4 * k + s + nrow, :
                            ].rearrange("p d h -> p (d h)")
                            nc.tensor.matmul(
                                pst[:, 512 * ki : 512 * ki + 128 * nrow],
                                bw,
                                rhs,
                                start=(s == 0),
                                stop=(s == 2),
                            )
                    nrows = 6 if k0 == 8 else 8
                    dst = t3_r[:, 4 * k0 : 4 * k0 + nrows, :].rearrange(
                        "p r h -> p (r h)"
                    )
                    evac(dst, pst[:, 0 : 128 * nrows])
                return emit
            return [mk(k0) for k0 in range(0, 10, 2)]

        def d_final(v, t3_r, eng):
            B = bxp.tile([HP, N_BOX], F16, tag=f"box{v}", name=f"box{v}")[:]
            B_r = B.rearrange("p (do h) -> p do h", h=HP)
            eng.tensor_add(B_r, t3_r[:, 0:DQ, :], t3_r[:, 3 : 3 + DQ, :])
            eng.tensor_add(B_r, B_r, t3_r[:, 6 : 6 + DQ, :])
            return B

        def pass2_chunks9(v, yt_r):
            """9-shift pass2: box filter D-combination entirely on the PE;
            evacuates the final box directly (no t3 / d_final)."""
            bw = bandl if v in ("I", "J") else bandq
            B = bxp.tile([HP, N_BOX], F16, tag=f"box{v}", name=f"box{v}")[:]
            boxes[v] = B
            B_r = B.rearrange("p (do h) -> p do h", h=HP)
            def mk(g0):
                def emit():
                    pst = ps1.tile([HP, 1024], F32, tag="ps1", name="ps9")[:]
                    for gi in range(2):
                        g = g0 + gi
                        for s in range(9):
                            rhs = yt_r[
                                :, 4 * g + s : 4 * g + s + 4, :
                            ].rearrange("p d h -> p (d h)")
                            nc.tensor.matmul(
                                pst[:, 512 * gi : 512 * (gi + 1)],
                                bw,
                                rhs,
                                start=(s == 0),
                                stop=(s == 8),
                            )
                    dst = B_r[:, 4 * g0 : 4 * g0 + 8, :].rearrange(
                        "p r h -> p (r h)"
                    )
                    evac(dst, pst)
                return emit
            return [mk(g0) for g0 in (0, 2, 4, 6)]

        # ---- smoothness Gram chunks (PE) ----
        def g_chunk(c, r_lo, r_hi, with_end):
            def emit():
                fl = fl_ap[c]
                for r in range(r_lo, r_hi):
                    rhs = fl[:, r : r + 2, :].rearrange("p d w -> p (d w)")
                    nc.tensor.matmul(
                        psG[:, 0:256],
                        fl[:, r, :],
                        rhs,
                        start=(c == 0 and r == 0),
                        stop=False,
                        skip_group_check=True,
                    )
                if with_end:
                    nc.tensor.matmul(
                        psG[:, 0:128], fl[:, 32, :], fl[:, 32, :],
                        start=False, stop=False, skip_group_check=True,
                    )
                    nc.tensor.matmul(
                        psG[:, 256:384], fl[:, 0, :], fl[:, 0, :],
                        start=False, stop=False, skip_group_check=True,
                    )
                    nc.tensor.matmul(
                        psG[:, 384:512], fl[:, 32, :], fl[:, 32, :],
                        start=False, stop=(c == 2), skip_group_check=True,
                    )
            return emit

        def gp_chunk(c, r_lo, r_hi):
            def emit():
                ft = flt_ap[c]
                for r in range(r_lo, r_hi):
                    nc.tensor.matmul(
                        psGp,
                        ft[:, r, :],
                        ft[:, r, :],
                        start=(c == 0 and r == 0),
                        stop=(c == 2 and r == FLOWT_D - 1),
                        skip_group_check=True,
                    )
            return emit

        extra_q = []
        if not NO_GRAM:
            for c in range(3):
                extra_q.append(g_chunk(c, 0, 16, False))
                extra_q.append(g_chunk(c, 16, 32, True))
            for c in range(3):
                extra_q.append(gp_chunk(c, 0, 16))
                extra_q.append(gp_chunk(c, 16, 32))
        extra_i = [0]

        def pop_extra():
            if extra_i[0] < len(extra_q):
                extra_q[extra_i[0]]()
                extra_i[0] += 1

        # ---- mse (GPSIMD) ----
        inJmid = inJ_r[:, 4 : 4 + DQ, :].rearrange("p d w -> p (d w)")

        def mse():
            for h in range(2):
                lo, hi = 2048 * h, 2048 * (h + 1)
                md = scp.tile([HP, 2048], F16, tag="tP", name=f"mse{h}")[:]
                md2 = scp.tile([HP, 2048], F16, tag="tQ", name=f"mse2{h}")[:]
                eng = nc.vector if NO_GPS else nc.gpsimd
                eng.tensor_sub(md, inJmid[:, lo:hi], recon[:, lo:hi])
                nc.scalar.activation(
                    md2, md, AF.Square,
                    accum_out=acc[:, COL_MSE + h : COL_MSE + h + 1],
                )

        # ---- software-pipelined emission ----
        boxes = {}
        yt_rs, t3_rs = {}, {}

        def begin_var(v):
            ytt = ytp.tile([HP, N_YT], F16, tag="yt", name=f"yt{v}")[:]
            yt_rs[v] = ytt.rearrange("p (d h) -> p d h", h=HP)
            return pass1_chunks(v, srcs[v], yt_rs[v])

        def begin_pass2(v):
            t3t = t3p.tile([HP, N_T3], F16, tag="t3", name=f"t3{v}")[:]
            t3_rs[v] = t3t.rearrange("p (r h) -> p r h", h=HP)
            return pass2_chunks(v, yt_rs[v], t3_rs[v])

        # cc intermediates overwrite dead box tiles in place (see cc math):
        #   post I:  (nothing)
        #   post II: m3 = bj^2 (ACT -> tmp3); m1 = bi*bj -> bxJ; m2 = bi^2 -> bxI
        #   in IJ-zip: Ivar = bii - m2 -> bxI; GPS JJ-prod done long before
        #   endgame slices: dfJJ -> bxJJ; Jvar -> bxJJ; den -> bxII;
        #                   lnd -> bxJJ; rcp = exp(-lnd) -> bxII
        #   tail slices: cross = bij - m1 -> bxJ; c2 -> bxIJ; cw -> bxJ; accum
        tmp3 = pp.tile([HP, N_BOX], F16, tag="tmp3", name="tmp3")[:]

        # GPS queue: prodII, prodJJ, d_final(J), mse subs
        gps = nc.vector if NO_GPS else nc.gpsimd

        def post_var(v):
            if v == "J":
                boxes["J"] = d_final("J", t3_rs["J"], gps)
            elif v == "I":
                boxes["I"] = d_final("I", t3_rs["I"], nc.vector)
            elif v == "II":
                boxes["II"] = d_final("II", t3_rs["II"], nc.vector)
                bj, bi = boxes["J"], boxes["I"]
                nc.scalar.activation(tmp3, bj, AF.Square)  # m3 = bj^2
                nc.vector.tensor_mul(bj, bj, bi)         # m1 = bi*bj -> bxJ
                nc.vector.tensor_mul(bi, bi, bi)         # m2 = bi^2  -> bxI

        pending_p2 = None
        pending_v = None
        prod_ahead = {"JJ": "IJ"}
        extras_on = {"II", "JJ", "IJ"}
        ij_extras = {}  # zip-step inserts for the v == "IJ" phase

        for v in VARS:
            p1 = begin_var(v)
            if pending_p2 is None:
                for e in p1:
                    e()
            else:
                ahead = prod_ahead.get(v)
                for ci in range(len(p1)):
                    if ci < len(pending_p2):
                        pending_p2[ci]()
                    p1[ci]()
                    if ahead and ci < 4:
                        product_chunk(ahead, ci, n=4)
                    if v in extras_on and ci >= 1:
                        pop_extra()
                    if v == "IJ" and ci in ij_extras:
                        ij_extras[ci]()
                post_var(pending_v)
            if v == "IJ":
                # during zip(p2_JJ, p1_IJ): Ivar + mse
                ij_extras = {}
            pending_p2 = begin_pass2(v)
            pending_v = v
        ij_p2 = pending_p2

        # Ivar + mse emitted at the start of the endgame (inputs ready by now)
        nc.vector.tensor_sub(boxes["I"], boxes["II"], boxes["I"])  # Ivar -> bxI
        mse()

        LASTV = VARS[-1]
        Blast = bxp.tile([HP, N_BOX], F16, tag=f"box{LASTV}", name=f"box{LASTV}")[:]
        Blast_r = Blast.rearrange("p (do h) -> p do h", h=HP)
        boxes[LASTV] = Blast
        bxJJ = bxp.tile([HP, N_BOX], F16, tag="boxJJ", name="boxJJ")[:]
        bxJJ_r = bxJJ.rearrange("p (do h) -> p do h", h=HP)
        boxes["JJ"] = bxJJ

        NROW_SL = DQ // NSL

        def jj_stage(sl):
            """Per-slice: d_final(JJ), Jvar, den, lnd, rcp."""
            do0 = NROW_SL * sl
            lo, hi = sl * NS, (sl + 1) * NS
            t3r = t3_rs["JJ"]
            nc.vector.tensor_add(
                bxJJ_r[:, do0 : do0 + NROW_SL, :],
                t3r[:, do0 : do0 + NROW_SL, :],
                t3r[:, do0 + 3 : do0 + 3 + NROW_SL, :],
            )
            nc.vector.tensor_add(
                bxJJ_r[:, do0 : do0 + NROW_SL, :],
                bxJJ_r[:, do0 : do0 + NROW_SL, :],
                t3r[:, do0 + 6 : do0 + 6 + NROW_SL, :],
            )
            bjj = bxJJ[:, lo:hi]
            ivar = boxes["I"][:, lo:hi]
            den_rcp = boxes["II"][:, lo:hi]
            nc.vector.tensor_sub(bjj, bjj, tmp3[:, lo:hi])   # Jvar
            nc.vector.tensor_mul(den_rcp, ivar, bjj)         # den -> bxII
            nc.scalar.activation(bjj, den_rcp, AF.Ln, bias=eps_ap)
            nc.scalar.activation(den_rcp, bjj, AF.Exp, scale=-1.0)  # rcp

        def ij_stage(sl):
            """Per-slice: d_final(IJ), cross, cc, accumulate."""
            do0 = NROW_SL * sl
            lo, hi = sl * NS, (sl + 1) * NS
            t3r = t3_rs[LASTV]
            nc.vector.tensor_add(
                Blast_r[:, do0 : do0 + NROW_SL, :],
                t3r[:, do0 : do0 + NROW_SL, :],
                t3r[:, do0 + 3 : do0 + 3 + NROW_SL, :],
            )
            nc.vector.tensor_add(
                Blast_r[:, do0 : do0 + NROW_SL, :],
                Blast_r[:, do0 : do0 + NROW_SL, :],
                t3r[:, do0 + 6 : do0 + 6 + NROW_SL, :],
            )
            m1 = boxes["J"][:, lo:hi]
            rcp = boxes["II"][:, lo:hi]
            bij = Blast[:, lo:hi]
            nc.vector.tensor_sub(m1, bij, m1)          # cross -> bxJ
            nc.vector.tensor_mul(bij, m1, m1)          # cross^2 -> bxIJ
            nc.vector.tensor_mul(m1, bij, rcp)         # cc -> bxJ
            nc.scalar.activation(
                m1, m1, AF.Copy,
                accum_out=acc[:, COL_CC + sl : COL_CC + sl + 1],
            )

        # endgame: interleave pass2(IJ) chunks with jj/ij slice stages
        ij_p2[0]()
        pop_extra()
        ij_p2[1]()
        jj_stage(0)
        ij_p2[2]()
        ij_stage(0)
        jj_stage(1)
        ij_p2[3]()
        ij_stage(1)
        jj_stage(2)
        ij_p2[4]()
        ij_stage(2)
        jj_stage(3)
        ij_stage(3)
        while extra_i[0] < len(extra_q):
            pop_extra()

        # smoothness masked reductions
        if not NO_GRAM:
            if NO_TTR:
                nc.vector.tensor_copy(gs1, psG)
                nc.vector.tensor_mul(gs1, gs1, masks[:, 0:512])
                nc.scalar.activation(
                    gs1, gs1, AF.Copy,
                    accum_out=acc[:, COL_SMG : COL_SMG + 1],
                )
                nc.vector.tensor_copy(gs2, psGp)
                nc.vector.tensor_mul(gs2, gs2, masks[:, 512:640])
                nc.scalar.activation(
                    gs2, gs2, AF.Copy,
                    accum_out=acc[:, COL_SMGP : COL_SMGP + 1],
                )
            else:
                nc.vector.tensor_tensor_reduce(
                    out=gs1, in0=psG, in1=masks[:, 0:512],
                    scale=1.0, scalar=0.0, op0=ALU.mult, op1=ALU.add,
                    accum_out=acc[:, COL_SMG : COL_SMG + 1],
                )
                nc.vector.tensor_tensor_reduce(
                    out=gs2, in0=psGp, in1=masks[:, 512:640],
                    scale=1.0, scalar=0.0, op0=ALU.mult, op1=ALU.add,
                    accum_out=acc[:, COL_SMGP : COL_SMGP + 1],
                )

        nc.sync.dma_start(out=d_out, in_=acc)

    nc.compile()
    return nc


def _make_consts():
    k = np.arange(HP)
    band = (np.abs(k[:, None] - k[None, :]) <= 4).astype(np.float16)
    bandq = (band * np.float16(S16)).astype(np.float16)
    bandl = (band * np.float16(TLIN)).astype(np.float16)

    # quadratic difference mask: diag [1,2,...,2,1], off-diag -1
    dg = np.full(HP, 2.0)
    dg[0] = 1.0
    dg[-1] = 1.0
    Mq = np.diag(dg)
    Mq += np.diag(np.full(HP - 1, -1.0), 1)
    Mq += np.diag(np.full(HP - 1, -1.0), -1)
    I = np.eye(HP)
    masks = np.zeros((HP, N_MASK), np.float16)
    masks[:, 0:128] = Mq + 2 * I          # on G
    masks[:, 128:256] = -2 * I            # on C
    masks[:, 256:384] = -I                # on A
    masks[:, 384:512] = -(I + Mq)         # on B
    masks[:, 512:640] = Mq                # on G'
    return band, bandq, bandl, masks


def _shard_inputs(imgsA, recon_A, warped_BA, flow_BA):
    bandh, bandq, bandl, masks = _make_consts()
    in_maps = []
    for core in range(8):
        b, q = divmod(core, 4)
        d0 = DQ * q

        def slab(vol):
            s = np.zeros((HP, D_IN, W), np.float16)
            lo, hi = d0 - 4, d0 + DQ + 4
            clo, chi = max(lo, 0), min(hi, D_FULL)
            s[:, clo - lo : chi - lo, :] = vol[clo:chi].transpose(1, 0, 2)
            return s.reshape(HP, N_IN)

        rec = (
            recon_A[b, 0, d0 : d0 + DQ]
            .transpose(1, 0, 2)
            .astype(np.float16)
            .reshape(HP, N_RECON)
        )

        fl = np.empty((HP, 3, FLOW_D, W), np.float16)
        hi = min(d0 + FLOW_D, D_FULL)
        n = hi - d0
        fl[:, :, :n] = flow_BA[b, :, d0:hi].transpose(2, 0, 1, 3)
        if n < FLOW_D:
            fl[:, :, n:] = fl[:, :, n - 1 : n]

        # transposed flow tiles: [w, (c, d, h)]
        flt = (
            flow_BA[b, :, d0 : d0 + DQ]
            .transpose(3, 0, 1, 2)
            .astype(np.float16)
        )

        in_maps.append(
            {
                "inI": slab(warped_BA[b, 0]),
                "inJ": slab(imgsA[b, 0]),
                "recon": np.ascontiguousarray(rec),
                "flow": np.ascontiguousarray(fl).reshape(HP, 3 * N_FLOW_C),
                "flowT": np.ascontiguousarray(flt).reshape(HP, 3 * N_FLOWT_C),
                "bandh": bandh,
                "bandq": bandq,
                "bandl": bandl,
                "masks": masks,
            }
        )
    return in_maps


def _install_profile_shim():
    """Wire up NTFF profiling under axon when antenv.axon_hooks is absent."""
    try:
        import antenv.axon_hooks  # noqa: F401

        return True
    except ImportError:
        pass
    import contextlib
    import ctypes
    import types

    so_path = "/opt/axon/libaxon_pjrt.so"
    if not os.path.exists(so_path):
        return False
    lib = ctypes.CDLL(so_path)
    if not hasattr(lib, "axon_start_nrt_profile"):
        return False
    lib.axon_start_nrt_profile.argtypes = [
        ctypes.POINTER(ctypes.c_int64),
        ctypes.c_size_t,
    ]
    lib.axon_start_nrt_profile.restype = ctypes.c_int64
    lib.axon_stop_nrt_profile.argtypes = [ctypes.c_char_p]
    lib.axon_stop_nrt_profile.restype = ctypes.c_int64

    @contextlib.contextmanager
    def _hook(output_dir, device_ids):
        import jax

        jax.devices()
        if device_ids:
            ids = (ctypes.c_int64 * len(device_ids))(*device_ids)
            rc = lib.axon_start_nrt_profile(ids, len(device_ids))
        else:
            rc = lib.axon_start_nrt_profile(None, 0)
        if rc != 0:
            raise RuntimeError(f"axon_start_nrt_profile rc={rc}")
        try:
            yield
        finally:
            n = lib.axon_stop_nrt_profile(str(output_dir).encode())
            print(f"ntff profile: {n} file(s) written to {output_dir}")

    mod = types.ModuleType("antenv.axon_hooks")
    mod.get_axon_ntff_profile_hook = lambda: _hook
    mod.set_axon_ntff_profile_hook = lambda h: None
    import antenv

    sys.modules["antenv.axon_hooks"] = mod
    antenv.axon_hooks = mod

    import concourse.bass_utils as _bu

    _bu.upload_artifacts = lambda tmpdir: tmpdir
    return True


LAST_EXEC_NS = None
LAST_RESULTS = None


def kernel(imgsA, recon_A, warped_BA, flow_BA):
    global LAST_EXEC_NS, LAST_RESULTS
    if "nc" not in _CACHE:
        _CACHE["nc"] = _build_program()
    nc = _CACHE["nc"]

    in_maps = _shard_inputs(
        np.asarray(imgsA, np.float32),
        np.asarray(recon_A, np.float32),
        np.asarray(warped_BA, np.float32),
        np.asarray(flow_BA, np.float32),
    )
    trace = os.environ.get("GVSL_TRACE", "0") == "1"
    if trace:
        trace = _install_profile_shim()
    tmpdir = os.environ.get("GVSL_TRACE_DIR") or None
    res = run_bass_kernel_spmd(
        nc, in_maps, core_ids=list(range(8)), trace=trace, tmpdir=tmpdir
    )
    LAST_EXEC_NS = res.exec_time_ns
    LAST_RESULTS = res

    cc = mse_s = smg = smgp = 0.0
    for r in res.results:
        o = np.asarray(r["out"], np.float64)
        cc += o[:, COL_CC : COL_CC + NSL].sum()
        mse_s += o[:, COL_MSE : COL_MSE + 4].sum()
        smg += o[:, COL_SMG].sum()
        smgp += o[:, COL_SMGP].sum()

    n_vox = 2 * 1 * 128 * 128 * 128
    n_d = 2 * 3 * 127 * 128 * 128
    ncc_loss = 1.0 - cc / n_vox
    mse_loss = mse_s / n_vox
    smooth_loss = (smg + smgp) / (3.0 * n_d)
    return (
        np.float32(ncc_loss),
        np.float32(mse_loss),
        np.float32(smooth_loss),
    )


# revision 30
# speedup vs baseline: 1.1268x; 1.0588x over previous
"""GVSL loss (NCC + MSE + smoothness) as a distributed Bass kernel on 8 TRN2 cores.

Sharding: batch(2) x depth-quarters(4) = 8 shards; each core owns a 32-deep
output slab (+4-voxel halo for the 9^3 box filter).

NCC box filter strategy (per var in {I, J, I^2, J^2, IJ}):
  pass1 (PE):  per d-row matmul(lhsT=V_d[h,w], rhs=BandH[h,h']) -> PSUM [w, h]
               = H-box + transpose in one shot (fp16, FD=128)
  evac1:       PSUM -> SBUF fp16 YT [w, (d, h)]   (rotating DVE/ACT)
  pass2 (PE):  stationary BandW (scaled); 3 d-shifted FD=512 matmuls accumulate
               -> t3[r] = Z[r]+Z[r+1]+Z[r+2] (W-box + D-triple), PSUM
  evac2:       PSUM -> SBUF fp16 T3
  D-final:     S = t3[d] + t3[d+3] + t3[d+6]  (GPSIMD for var J, DVE otherwise)

cc math: cc = (crossS * rsqrt(IvarS*JvarS + eps))^2 -- Rsqrt+Square+Copy all
live in the reciprocal_sqrt_and_small ACT table set: one table load total.

Smoothness loss entirely on the PE via Gram matrices: for flow tiles
X_r = [h, w] per (channel, depth-row),
  sum dz^2 (w-diff)  = <Mq, G>        with G  = sum X_r^T X_r
  sum dy^2 (d-diff)  = <2I, G> - <I, A> - <I, B> - 2<I, C>,
                       C = sum X_r^T X_{r+1}, A = sum_c G_0, B = sum_c G_32
  sum dx^2 (h-diff)  = <Mq, G'>       with G' = sum X'_r^T X'_r (transposed)
All Gram matmuls accumulate into two PSUM banks; masked reduction is a single
fused DVE tensor_tensor_reduce per bank. Mq = quadratic diff mask
(diag [1,2,...,2,1], off-diag -1).

MSE: GPSIMD sub + fused scalar_tensor_tensor square-accumulate.
"""

import os
import sys

for _p in ("/opt/trn_rl_repo",):
    if _p not in sys.path:
        sys.path.insert(0, _p)

import numpy as np

import concourse.bass as bass
import concourse.tile as tile
from concourse import bacc, mybir
from concourse.bass_utils import run_bass_kernel_spmd

F32 = mybir.dt.float32
F16 = mybir.dt.float16
AF = mybir.ActivationFunctionType
ALU = mybir.AluOpType

HP = 128
W = 128
D_FULL = 128
DQ = 32
D_IN = DQ + 8     # 40 slab rows incl halo
YT_R = 40
T3_R = 40
FLOW_D = DQ + 1   # 33 rows for untransposed flow (d-pairs)
FLOWT_D = DQ      # 32 rows for transposed flow

N_IN = D_IN * W           # 5120
N_YT = YT_R * HP          # 5120
N_T3 = T3_R * HP          # 5120
N_BOX = DQ * HP           # 4096
N_RECON = DQ * W          # 4096
N_FLOW_C = FLOW_D * W     # 4224
N_FLOWT_C = FLOWT_D * HP  # 4096
N_MASK = 512 + 128        # 640

S16 = float(np.float16(1.0 / 27.0))          # quadratic-var scale
TLIN = float(np.float16(np.sqrt(S16 / 729.0)))  # linear-var scale
EPS_S = 1e-5 * S16 * S16

NSL = 4                   # cc slices
NS = N_BOX // NSL         # 1024 els per slice

COL_CC = 0     # +NSL
COL_MSE = COL_CC + NSL   # +4
COL_SMG = COL_MSE + 4
COL_SMGP = COL_SMG + 1
ACC_W = 12

VARS = ("J", "I", "II", "JJ", "IJ")

# evacuation engine rotation (PSUM -> SBUF copies): ACT-heavy
EVAC_PAT = ("scalar", "scalar", "scalar", "vector")

_CACHE = {}

# bisect/debug knobs
NO_GRAM = os.environ.get("GVSL_NO_GRAM", "0") == "1"   # skip smooth Gram MMs + TTRs
NO_TTR = os.environ.get("GVSL_NO_TTR", "1") == "1"     # tensor_tensor_reduce hangs TRN2 HW; keep fallback
NO_GPS = os.environ.get("GVSL_NO_GPS", "0") == "1"     # no gpsimd work (all on DVE)


def _patch_act_tables():
    """Reorder activation-table sets so natural_log_exp_and_others (which
    contains ln + exp + square + copy) is preferred: one ACT table load."""
    from concourse import hw_specs

    if getattr(hw_specs, "_gvsl_patched", False):
        return
    orig = hw_specs.get_activation_tables

    def patched(arch):
        t = dict(orig(arch))
        key = "natural_log_exp_and_others"
        if key in t:
            t = {key: t[key], **{k: v for k, v in t.items() if k != key}}
        return t

    hw_specs.get_activation_tables = patched
    bacc.get_activation_tables = patched
    hw_specs._gvsl_patched = True


def _build_program():
    if os.environ.get("GVSL_PATCH_TABLES", "0") == "1":
        _patch_act_tables()  # WARNING: hangs TRN2 HW (table id remap mismatch)
    nc = bacc.Bacc("TRN2", target_bir_lowering=False, debug=False, num_devices=8)

    d_inI = nc.dram_tensor("inI", [HP, N_IN], F16, kind="ExternalInput").ap()
    d_inJ = nc.dram_tensor("inJ", [HP, N_IN], F16, kind="ExternalInput").ap()
    d_recon = nc.dram_tensor("recon", [HP, N_RECON], F16, kind="ExternalInput").ap()
    d_flow = nc.dram_tensor("flow", [HP, 3 * N_FLOW_C], F16, kind="ExternalInput").ap()
    d_flowT = nc.dram_tensor(
        "flowT", [HP, 3 * N_FLOWT_C], F16, kind="ExternalInput"
    ).ap()
    d_bandh = nc.dram_tensor("bandh", [HP, HP], F16, kind="ExternalInput").ap()
    d_bandq = nc.dram_tensor("bandq", [HP, HP], F16, kind="ExternalInput").ap()
    d_bandl = nc.dram_tensor("bandl", [HP, HP], F16, kind="ExternalInput").ap()
    d_masks = nc.dram_tensor("masks", [HP, N_MASK], F16, kind="ExternalInput").ap()
    d_out = nc.dram_tensor("out", [HP, ACC_W], F32, kind="ExternalOutput").ap()

    from contextlib import ExitStack

    with tile.TileContext(nc) as tc, ExitStack() as es:
        pp = es.enter_context(tc.tile_pool(name="persist", bufs=1))
        prp = es.enter_context(tc.tile_pool(name="prodp", bufs=1))
        ytp = es.enter_context(tc.tile_pool(name="ytp", bufs=2))
        t3p = es.enter_context(tc.tile_pool(name="t3p", bufs=2))
        bxp = es.enter_context(tc.tile_pool(name="boxp", bufs=1))
        scp = es.enter_context(tc.tile_pool(name="scrp", bufs=1))
        flp = es.enter_context(tc.tile_pool(name="flowp", bufs=3))
        ps1 = es.enter_context(tc.tile_pool(name="psum1", bufs=3, space="PSUM"))
        psg = es.enter_context(tc.tile_pool(name="psumG", bufs=1, space="PSUM"))

        acc = pp.tile([HP, ACC_W], F32, tag="acc", name="acc")[:]
        nc.gpsimd.memset(acc, 0.0)
        eps_ap = pp.tile([HP, 1], F32, tag="epsc", name="epsc")[:]
        nc.gpsimd.memset(eps_ap, EPS_S)

        bandh = pp.tile([HP, HP], F16, tag="bandh", name="bandh")[:]
        bandq = pp.tile([HP, HP], F16, tag="bandq", name="bandq")[:]
        bandl = pp.tile([HP, HP], F16, tag="bandl", name="bandl")[:]
        masks = pp.tile([HP, N_MASK], F16, tag="masks", name="masks")[:]
        inI = pp.tile([HP, N_IN], F16, tag="inI", name="inI")[:]
        inJ = pp.tile([HP, N_IN], F16, tag="inJ", name="inJ")[:]
        recon = pp.tile([HP, N_RECON], F16, tag="recon", name="recon")[:]
        gs1 = pp.tile([HP, 512], F32, tag="gs1", name="gs1")[:]
        gs2 = pp.tile([HP, HP], F32, tag="gs2", name="gs2")[:]

        # PSUM Gram accumulators: [G | C | A | B] and G'
        psG = psg.tile([HP, 512], F32, tag="G", name="psG")[:]
        psGp_full = psg.tile([HP, 512], F32, tag="Gp", name="psGp")[:]
        psGp = psGp_full[:, 0:HP]

        NQ = N_IN // 4
        for q in range(4):
            nc.sync.dma_start(
                out=inJ[:, NQ * q : NQ * (q + 1)],
                in_=d_inJ[:, NQ * q : NQ * (q + 1)],
            )
        nc.sync.dma_start(out=bandh, in_=d_bandh)
        for q in range(4):
            nc.sync.dma_start(
                out=inI[:, NQ * q : NQ * (q + 1)],
                in_=d_inI[:, NQ * q : NQ * (q + 1)],
            )
        nc.sync.dma_start(out=bandl, in_=d_bandl)
        nc.sync.dma_start(out=bandq, in_=d_bandq)
        nc.sync.dma_start(out=masks, in_=d_masks)

        # flow channels stream in per-channel ring buffers
        fl_ap = {}
        flt_ap = {}
        for c in range(3):
            t = flp.tile([HP, N_FLOW_C], F16, tag="flc", name=f"flc{c}")[:]
            nc.sync.dma_start(
                out=t, in_=d_flow[:, c * N_FLOW_C : (c + 1) * N_FLOW_C]
            )
            fl_ap[c] = t.rearrange("p (d w) -> p d w", w=W)
        for c in range(3):
            t = flp.tile([HP, N_FLOWT_C], F16, tag="flt", name=f"flt{c}")[:]
            nc.sync.dma_start(
                out=t, in_=d_flowT[:, c * N_FLOWT_C : (c + 1) * N_FLOWT_C]
            )
            flt_ap[c] = t.rearrange("p (d h) -> p d h", h=HP)
        nc.sync.dma_start(out=recon, in_=d_recon)

        inI_r = inI.rearrange("p (d w) -> p d w", w=W)
        inJ_r = inJ.rearrange("p (d w) -> p d w", w=W)

        evac_ct = [0]

        def evac(dst, src):
            eng = EVAC_PAT[evac_ct[0] % len(EVAC_PAT)]
            if eng == "vector":
                nc.vector.tensor_copy(dst, src)
            else:
                nc.scalar.copy(dst, src)
            evac_ct[0] += 1

        # ---- products: JJ full on GPSIMD; IJ, II chunked on DVE ----
        prod_tiles = {}
        srcs = {"J": inJ_r, "I": inI_r}
        for v in ("JJ", "IJ", "II"):
            prod_tiles[v] = prp.tile(
                [HP, N_IN], F16, tag=f"prod{v}", name=f"prod{v}"
            )[:]
            srcs[v] = prod_tiles[v].rearrange("p (d w) -> p d w", w=W)
        peng = nc.vector if NO_GPS else nc.gpsimd
        peng.tensor_mul(prod_tiles["II"], inI, inI)
        peng.tensor_mul(prod_tiles["JJ"], inJ, inJ)

        def product_chunk(v, c, n=2):
            lo = (N_IN // n) * c
            hi = (N_IN // n) * (c + 1)
            a = inJ if v == "JJ" else inI
            b = inJ if v in ("JJ", "IJ") else inI
            nc.vector.tensor_mul(prod_tiles[v][:, lo:hi], a[:, lo:hi], b[:, lo:hi])

        # ---- pass1 / pass2 ----
        def pass1_chunks(v, src_r, yt_r):
            def mk(g0):
                def emit():
                    pst = ps1.tile([HP, 1024], F32, tag="ps1", name="ps1")[:]
                    for q in range(8):
                        nc.tensor.matmul(
                            pst[:, 128 * q : 128 * (q + 1)],
                            src_r[:, g0 + q, :],
                            bandh,
                            start=True,
                            stop=True,
                        )
                    dst = yt_r[:, g0 : g0 + 8, :].rearrange("p d h -> p (d h)")
                    evac(dst, pst)
                return emit
            return [mk(g0) for g0 in range(0, D_IN, 8)]

        def pass2_chunks(v, yt_r, t3_r):
            bw = bandl if v in ("I", "J") else bandq
            def mk(k0):
                def emit():
                    pst = ps1.tile([HP, 1024], F32, tag="ps1", name="ps2")[:]
                    for ki in range(2):
                        k = k0 + ki
                        nrow = 2 if k == 9 else 4
                        for s in range(3):
                            rhs = yt_r[
                                :, 4 * k + s : 4 * k + s + nrow, :
                            ].rearrange("p d h -> p (d h)")
                            nc.tensor.matmul(
                                pst[:, 512 * ki : 512 * ki + 128 * nrow],
                                bw,
                                rhs,
                                start=(s == 0),
                                stop=(s == 2),
                            )
                    nrows = 6 if k0 == 8 else 8
                    dst = t3_r[:, 4 * k0 : 4 * k0 + nrows, :].rearrange(
                        "p r h -> p (r h)"
                    )
                    evac(dst, pst[:, 0 : 128 * nrows])
                return emit
            return [mk(k0) for k0 in range(0, 10, 2)]

        def d_final(v, t3_r, eng):
            B = bxp.tile([HP, N_BOX], F16, tag=f"box{v}", name=f"box{v}")[:]
            B_r = B.rearrange("p (do h) -> p do h", h=HP)
            eng.tensor_add(B_r, t3_r[:, 0:DQ, :], t3_r[:, 3 : 3 + DQ, :])
            eng.tensor_add(B_r, B_r, t3_r[:, 6 : 6 + DQ, :])
            return B

        def pass2_chunks9(v, yt_r):
            """9-shift pass2: box filter D-combination entirely on the PE;
            evacuates the final box directly (no t3 / d_final)."""
            bw = bandl if v in ("I", "J") else bandq
            B = bxp.tile([HP, N_BOX], F16, tag=f"box{v}", name=f"box{v}")[:]
            boxes[v] = B
            B_r = B.rearrange("p (do h) -> p do h", h=HP)
            def mk(g0):
                def emit():
                    pst = ps1.tile([HP, 1024], F32, tag="ps1", name="ps9")[:]
                    for gi in range(2):
                        g = g0 + gi
                        for s in range(9):
                            rhs = yt_r[
                                :, 4 * g + s : 4 * g + s + 4, :
                            ].rearrange("p d h -> p (d h)")
                            nc.tensor.matmul(
                                pst[:, 512 * gi : 512 * (gi + 1)],
                                bw,
                                rhs,
                                start=(s == 0),
                                stop=(s == 8),
                            )
                    dst = B_r[:, 4 * g0 : 4 * g0 + 8, :].rearrange(
                        "p r h -> p (r h)"
                    )
                    evac(dst, pst)
                return emit
            return [mk(g0) for g0 in (0, 2, 4, 6)]

        # ---- smoothness Gram chunks (PE) ----
        def g_chunk(c, r_lo, r_hi, with_end):
            def emit():
                fl = fl_ap[c]
                for r in range(r_lo, r_hi):
                    rhs = fl[:, r : r + 2, :].rearrange("p d w -> p (d w)")
                    nc.tensor.matmul(
                        psG[:, 0:256],
                        fl[:, r, :],
                        rhs,
                        start=(c == 0 and r == 0),
                        stop=False,
                        skip_group_check=True,
                    )
                if with_end:
                    nc.tensor.matmul(
                        psG[:, 0:128], fl[:, 32, :], fl[:, 32, :],
                        start=False, stop=False, skip_group_check=True,
                    )
                    nc.tensor.matmul(
                        psG[:, 256:384], fl[:, 0, :], fl[:, 0, :],
                        start=False, stop=False, skip_group_check=True,
                    )
                    nc.tensor.matmul(
                        psG[:, 384:512], fl[:, 32, :], fl[:, 32, :],
                        start=False, stop=(c == 2), skip_group_check=True,
                    )
            return emit

        def gp_chunk(c, r_lo, r_hi):
            def emit():
                ft = flt_ap[c]
                for r in range(r_lo, r_hi):
                    nc.tensor.matmul(
                        psGp,
                        ft[:, r, :],
                        ft[:, r, :],
                        start=(c == 0 and r == 0),
                        stop=(c == 2 and r == FLOWT_D - 1),
                        skip_group_check=True,
                    )
            return emit

        extra_q = []
        if not NO_GRAM:
            for c in range(3):
                extra_q.append(g_chunk(c, 0, 16, False))
                extra_q.append(g_chunk(c, 16, 32, True))
            for c in range(3):
                extra_q.append(gp_chunk(c, 0, 16))
                extra_q.append(gp_chunk(c, 16, 32))
        extra_i = [0]

        def pop_extra():
            if extra_i[0] < len(extra_q):
                extra_q[extra_i[0]]()
                extra_i[0] += 1

        # ---- mse (GPSIMD) ----
        inJmid = inJ_r[:, 4 : 4 + DQ, :].rearrange("p d w -> p (d w)")

        def mse():
            for h in range(2):
                lo, hi = 2048 * h, 2048 * (h + 1)
                md = scp.tile([HP, 2048], F16, tag="tP", name=f"mse{h}")[:]
                md2 = scp.tile([HP, 2048], F16, tag="tQ", name=f"mse2{h}")[:]
                eng = nc.vector if NO_GPS else nc.gpsimd
                eng.tensor_sub(md, inJmid[:, lo:hi], recon[:, lo:hi])
                nc.scalar.activation(
                    md2, md, AF.Square,
                    accum_out=acc[:, COL_MSE + h : COL_MSE + h + 1],
                )

        # ---- software-pipelined emission ----
        boxes = {}
        yt_rs, t3_rs = {}, {}

        def begin_var(v):
            ytt = ytp.tile([HP, N_YT], F16, tag="yt", name=f"yt{v}")[:]
            yt_rs[v] = ytt.rearrange("p (d h) -> p d h", h=HP)
            return pass1_chunks(v, srcs[v], yt_rs[v])

        def begin_pass2(v):
            t3t = t3p.tile([HP, N_T3], F16, tag="t3", name=f"t3{v}")[:]
            t3_rs[v] = t3t.rearrange("p (r h) -> p r h", h=HP)
            return pass2_chunks(v, yt_rs[v], t3_rs[v])

        # cc intermediates overwrite dead box tiles in place (see cc math):
        #   post I:  (nothing)
        #   post II: m3 = bj^2 (ACT -> tmp3); m1 = bi*bj -> bxJ; m2 = bi^2 -> bxI
        #   in IJ-zip: Ivar = bii - m2 -> bxI; GPS JJ-prod done long before
        #   endgame slices: dfJJ -> bxJJ; Jvar -> bxJJ; den -> bxII;
        #                   lnd -> bxJJ; rcp = exp(-lnd) -> bxII
        #   tail slices: cross = bij - m1 -> bxJ; c2 -> bxIJ; cw -> bxJ; accum
        tmp3 = pp.tile([HP, N_BOX], F16, tag="tmp3", name="tmp3")[:]

        # GPS queue: prodII, prodJJ, d_final(J), mse subs
        gps = nc.vector if NO_GPS else nc.gpsimd

        def post_var(v):
            if v == "J":
                boxes["J"] = d_final("J", t3_rs["J"], gps)
            elif v == "I":
                boxes["I"] = d_final("I", t3_rs["I"], nc.vector)
            elif v == "II":
                boxes["II"] = d_final("II", t3_rs["II"], nc.vector)
                bj, bi = boxes["J"], boxes["I"]
                nc.scalar.activation(tmp3, bj, AF.Square)  # m3 = bj^2
                nc.vector.tensor_mul(bj, bj, bi)         # m1 = bi*bj -> bxJ
                nc.vector.tensor_mul(bi, bi, bi)         # m2 = bi^2  -> bxI

        pending_p2 = None
        pending_v = None
        prod_ahead = {"JJ": "IJ"}
        extras_on = {"II", "JJ"}

        for v in VARS[:-1]:
            p1 = begin_var(v)
            if pending_p2 is None:
                for e in p1:
                    e()
            else:
                ahead = prod_ahead.get(v)
                for ci in range(len(p1)):
                    if ci < len(pending_p2):
                        pending_p2[ci]()
                    p1[ci]()
                    if ahead and ci < 4:
                        product_chunk(ahead, ci, n=4)
                    if v in extras_on and ci >= 1:
                        pop_extra()
                post_var(pending_v)
            pending_p2 = begin_pass2(v)
            pending_v = v

        LASTV = VARS[-1]
        Blast = bxp.tile([HP, N_BOX], F16, tag=f"box{LASTV}", name=f"box{LASTV}")[:]
        Blast_r = Blast.rearrange("p (do h) -> p do h", h=HP)
        boxes[LASTV] = Blast
        bxJJ = bxp.tile([HP, N_BOX], F16, tag="boxJJ", name="boxJJ")[:]
        bxJJ_r = bxJJ.rearrange("p (do h) -> p do h", h=HP)
        boxes["JJ"] = bxJJ

        NROW_SL = DQ // NSL

        def jj_dve(sl):
            """Per-slice DVE: d_final(JJ), Jvar, den."""
            do0 = NROW_SL * sl
            lo, hi = sl * NS, (sl + 1) * NS
            t3r = t3_rs["JJ"]
            nc.vector.tensor_add(
                bxJJ_r[:, do0 : do0 + NROW_SL, :],
                t3r[:, do0 : do0 + NROW_SL, :],
                t3r[:, do0 + 3 : do0 + 3 + NROW_SL, :],
            )
            nc.vector.tensor_add(
                bxJJ_r[:, do0 : do0 + NROW_SL, :],
                bxJJ_r[:, do0 : do0 + NROW_SL, :],
                t3r[:, do0 + 6 : do0 + 6 + NROW_SL, :],
            )
            bjj = bxJJ[:, lo:hi]
            nc.vector.tensor_sub(bjj, bjj, tmp3[:, lo:hi])   # Jvar
            nc.vector.tensor_mul(
                boxes["II"][:, lo:hi], boxes["I"][:, lo:hi], bjj
            )                                                # den -> bxII

        # last var: zip(p2_JJ, p1_IJ) with Ivar + jj-side slice inserts
        p1 = begin_var("IJ")
        for ci in range(len(p1)):
            pending_p2[ci]()
            p1[ci]()
            if ci >= 1:
                pop_extra()
            if ci == 1:
                nc.vector.tensor_sub(
                    boxes["I"], boxes["II"], boxes["I"]
                )                                            # Ivar -> bxI
            elif ci >= 2:
                jj_dve(ci - 2)
        ij_p2 = begin_pass2("IJ")

        def ij_stage(sl):
            """Per-slice: d_final(IJ), cross, cc, accumulate."""
            do0 = NROW_SL * sl
            lo, hi = sl * NS, (sl + 1) * NS
            t3r = t3_rs[LASTV]
            nc.vector.tensor_add(
                Blast_r[:, do0 : do0 + NROW_SL, :],
                t3r[:, do0 : do0 + NROW_SL, :],
                t3r[:, do0 + 3 : do0 + 3 + NROW_SL, :],
            )
            nc.vector.tensor_add(
                Blast_r[:, do0 : do0 + NROW_SL, :],
                Blast_r[:, do0 : do0 + NROW_SL, :],
                t3r[:, do0 + 6 : do0 + 6 + NROW_SL, :],
            )
            m1 = boxes["J"][:, lo:hi]
            rcp = boxes["II"][:, lo:hi]
            bij = Blast[:, lo:hi]
            nc.vector.tensor_sub(m1, bij, m1)          # cross -> bxJ
            nc.vector.tensor_mul(bij, m1, m1)          # cross^2 -> bxIJ
            nc.vector.tensor_mul(m1, bij, rcp)         # cc -> bxJ
            nc.scalar.activation(
                m1, m1, AF.Copy,
                accum_out=acc[:, COL_CC + sl : COL_CC + sl + 1],
            )

        # endgame: interleave pass2(IJ) chunks with the jj-side slice work;
        # Ln/Exp run as two full-size batched calls (one ACT table switch
        # each way instead of per-slice thrash), then the sliced cc tail.
        ij_p2[0]()
        jj_dve(3)
        ij_p2[1]()
        nc.scalar.activation(bxJJ, boxes["II"], AF.Ln, bias=eps_ap)  # lnd
        ij_p2[2]()
        nc.scalar.activation(boxes["II"], bxJJ, AF.Exp, scale=-1.0)  # rcp
        ij_p2[3]()
        mse()
        ij_p2[4]()
        while extra_i[0] < len(extra_q):
            pop_extra()
        for sl in range(NSL):
            ij_stage(sl)

        # smoothness masked reductions
        if not NO_GRAM:
            if NO_TTR:
                nc.vector.tensor_copy(gs1, psG)
                nc.vector.tensor_mul(gs1, gs1, masks[:, 0:512])
                nc.scalar.activation(
                    gs1, gs1, AF.Copy,
                    accum_out=acc[:, COL_SMG : COL_SMG + 1],
                )
                nc.vector.tensor_copy(gs2, psGp)
                nc.vector.tensor_mul(gs2, gs2, masks[:, 512:640])
                nc.scalar.activation(
                    gs2, gs2, AF.Copy,
                    accum_out=acc[:, COL_SMGP : COL_SMGP + 1],
                )
            else:
                nc.vector.tensor_tensor_reduce(
                    out=gs1, in0=psG, in1=masks[:, 0:512],
                    scale=1.0, scalar=0.0, op0=ALU.mult, op1=ALU.add,
                    accum_out=acc[:, COL_SMG : COL_SMG + 1],
                )
                nc.vector.tensor_tensor_reduce(
                    out=gs2, in0=psGp, in1=masks[:, 512:640],
                    scale=1.0, scalar=0.0, op0=ALU.mult, op1=ALU.add,
                    accum_out=acc[:, COL_SMGP : COL_SMGP + 1],
                )

        nc.sync.dma_start(out=d_out, in_=acc)

    nc.compile()
    return nc


def _make_consts():
    k = np.arange(HP)
    band = (np.abs(k[:, None] - k[None, :]) <= 4).astype(np.float16)
    bandq = (band * np.float16(S16)).astype(np.float16)
    bandl = (band * np.float16(TLIN)).astype(np.float16)

    # quadratic difference mask: diag [1,2,...,2,1], off-diag -1
    dg = np.full(HP, 2.0)
    dg[0] = 1.0
    dg[-1] = 1.0
    Mq = np.diag(dg)
    Mq += np.diag(np.full(HP - 1, -1.0), 1)
    Mq += np.diag(np.full(HP - 1, -1.0), -1)
    I = np.eye(HP)
    masks = np.zeros((HP, N_MASK), np.float16)
    masks[:, 0:128] = Mq + 2 * I          # on G
    masks[:, 128:256] = -2 * I            # on C
    masks[:, 256:384] = -I                # on A
    masks[:, 384:512] = -(I + Mq)         # on B
    masks[:, 512:640] = Mq                # on G'
    return band, bandq, bandl, masks


def _shard_inputs(imgsA, recon_A, warped_BA, flow_BA):
    bandh, bandq, bandl, masks = _make_consts()
    in_maps = []
    for core in range(8):
        b, q = divmod(core, 4)
        d0 = DQ * q

        def slab(vol):
            s = np.zeros((HP, D_IN, W), np.float16)
            lo, hi = d0 - 4, d0 + DQ + 4
            clo, chi = max(lo, 0), min(hi, D_FULL)
            s[:, clo - lo : chi - lo, :] = vol[clo:chi].transpose(1, 0, 2)
            return s.reshape(HP, N_IN)

        rec = (
            recon_A[b, 0, d0 : d0 + DQ]
            .transpose(1, 0, 2)
            .astype(np.float16)
            .reshape(HP, N_RECON)
        )

        fl = np.empty((HP, 3, FLOW_D, W), np.float16)
        hi = min(d0 + FLOW_D, D_FULL)
        n = hi - d0
        fl[:, :, :n] = flow_BA[b, :, d0:hi].transpose(2, 0, 1, 3)
        if n < FLOW_D:
            fl[:, :, n:] = fl[:, :, n - 1 : n]

        # transposed flow tiles: [w, (c, d, h)]
        flt = (
            flow_BA[b, :, d0 : d0 + DQ]
            .transpose(3, 0, 1, 2)
            .astype(np.float16)
        )

        in_maps.append(
            {
                "inI": slab(warped_BA[b, 0]),
                "inJ": slab(imgsA[b, 0]),
                "recon": np.ascontiguousarray(rec),
                "flow": np.ascontiguousarray(fl).reshape(HP, 3 * N_FLOW_C),
                "flowT": np.ascontiguousarray(flt).reshape(HP, 3 * N_FLOWT_C),
                "bandh": bandh,
                "bandq": bandq,
                "bandl": bandl,
                "masks": masks,
            }
        )
    return in_maps


def _install_profile_shim():
    """Wire up NTFF profiling under axon when antenv.axon_hooks is absent."""
    try:
        import antenv.axon_hooks  # noqa: F401

        return True
    except ImportError:
        pass
    import contextlib
    import ctypes
    import types

    so_path = "/opt/axon/libaxon_pjrt.so"
    if not os.path.exists(so_path):
        return False
    lib = ctypes.CDLL(so_path)
    if not hasattr(lib, "axon_start_nrt_profile"):
        return False
    lib.axon_start_nrt_profile.argtypes = [
        ctypes.POINTER(ctypes.c_int64),
        ctypes.c_size_t,
    ]
    lib.axon_start_nrt_profile.restype = ctypes.c_int64
    lib.axon_stop_nrt_profile.argtypes = [ctypes.c_char_p]
    lib.axon_stop_nrt_profile.restype = ctypes.c_int64

    @contextlib.contextmanager
    def _hook(output_dir, device_ids):
        import jax

        jax.devices()
        if device_ids:
            ids = (ctypes.c_int64 * len(device_ids))(*device_ids)
            rc = lib.axon_start_nrt_profile(ids, len(device_ids))
        else:
            rc = lib.axon_start_nrt_profile(None, 0)
        if rc != 0:
            raise RuntimeError(f"axon_start_nrt_profile rc={rc}")
        try:
            yield
        finally:
            n = lib.axon_stop_nrt_profile(str(output_dir).encode())
            print(f"ntff profile: {n} file(s) written to {output_dir}")

    mod = types.ModuleType("antenv.axon_hooks")
    mod.get_axon_ntff_profile_hook = lambda: _hook
    mod.set_axon_ntff_profile_hook = lambda h: None
    import antenv

    sys.modules["antenv.axon_hooks"] = mod
    antenv.axon_hooks = mod

    import concourse.bass_utils as _bu

    _bu.upload_artifacts = lambda tmpdir: tmpdir
    return True


LAST_EXEC_NS = None
LAST_RESULTS = None


def kernel(imgsA, recon_A, warped_BA, flow_BA):
    global LAST_EXEC_NS, LAST_RESULTS
    if "nc" not in _CACHE:
        _CACHE["nc"] = _build_program()
    nc = _CACHE["nc"]

    in_maps = _shard_inputs(
        np.asarray(imgsA, np.float32),
        np.asarray(recon_A, np.float32),
        np.asarray(warped_BA, np.float32),
        np.asarray(flow_BA, np.float32),
    )
    trace = os.environ.get("GVSL_TRACE", "0") == "1"
    if trace:
        trace = _install_profile_shim()
    tmpdir = os.environ.get("GVSL_TRACE_DIR") or None
    res = run_bass_kernel_spmd(
        nc, in_maps, core_ids=list(range(8)), trace=trace, tmpdir=tmpdir
    )
    LAST_EXEC_NS = res.exec_time_ns
    LAST_RESULTS = res

    cc = mse_s = smg = smgp = 0.0
    for r in res.results:
        o = np.asarray(r["out"], np.float64)
        cc += o[:, COL_CC : COL_CC + NSL].sum()
        mse_s += o[:, COL_MSE : COL_MSE + 4].sum()
        smg += o[:, COL_SMG].sum()
        smgp += o[:, COL_SMGP].sum()

    n_vox = 2 * 1 * 128 * 128 * 128
    n_d = 2 * 3 * 127 * 128 * 128
    ncc_loss = 1.0 - cc / n_vox
    mse_loss = mse_s / n_vox
    smooth_loss = (smg + smgp) / (3.0 * n_d)
    return (
        np.float32(ncc_loss),
        np.float32(mse_loss),
        np.float32(smooth_loss),
    )
